# revision 1
# baseline (speedup 1.0000x reference)
"""Paged GQA chunked-prefill attention for 8 Trainium2 NeuronCores.

Problem (hardcoded): B=4 seqs x Q=256 new tokens, H=32 query heads, KVH=8 kv
heads (GQA group G=4), D=128 head dim, paged KV cache of 512 blocks x 16
tokens, per-seq lengths in seq_lens (clamped to >= Q), causal masking.

Sharding: tensor-parallel over heads. Core h gets kv head h and query heads
h*4..h*4+3; block_tables/seq_lens are resolved host-side while packing the
shards; the output is all-gathered host-side over the hidden dim.

Per-core device kernel (seq b, kv chunk c of 128 positions, q = (g,t) -> 1024
columns, processed in two 512-column halves n):
  S^T[kv,qh] = K_c^T q            (f32r matmul, full PE rate)
  S^T += causal mask              (identity-lhsT matmul into the same bank)
  U = exp(SCALE * S^T)            (ScalarE, PSUM->SBUF, float32r out)
  l[2,qh] += ones2^T @ U          (wide denominator matmul, q stays on free)
  O^T[d,qh] += V_c^T @ U          (PSUM accumulation over chunks)
Per-seq epilogue: l -> SBUF (ScalarE), PE-transpose l to [128,8] and O^T to
[q,d], rlt = 1/l (VectorE), out = O * rlt (tensor_scalar), DMA out.

Sequences are processed longest-first so the PE warms up on the big unmasked
run while the remaining DMAs and the mask constants stream in.
"""
import math

import ml_dtypes
import numpy as np

import concourse.mybir as mybir
import concourse.tile as tile
from concourse import bacc
from concourse.bass_utils import run_bass_kernel_spmd

B, Q, H, D = 4, 256, 32, 128
KVH = 8
G = H // KVH
BLOCK = 16
NB = 128
KV = NB * BLOCK
NUM_BLOCKS = B * NB
SCALE = 1.0 / math.sqrt(D)
N_CORES = 8
CHUNK = 128
QCOLS = G * Q  # 1024 q columns per sequence per core
NHALF = 512

F32 = mybir.dt.float32
F32R = mybir.dt.float32r
NEG = -1.0e9


def _plan(seq_lens):
    """Per-seq chunk counts, offsets, and boundary-chunk mask tiles."""
    L = np.maximum(np.asarray(seq_lens, dtype=np.int64), Q)
    cb = [int((int(Lb) + CHUNK - 1) // CHUNK) for Lb in L]
    offs = np.concatenate([[0], np.cumsum(cb)]).astype(int)
    masked = []  # list of (b, c, mask[128,256])
    t = np.arange(Q)
    p = np.arange(CHUNK)
    for b in range(B):
        Lb = int(L[b])
        for c in range(cb[b]):
            if c * CHUNK + CHUNK - 1 > Lb - Q:
                kvpos = c * CHUNK + p
                m = np.where(
                    kvpos[:, None] > (Lb - Q) + t[None, :], NEG, 0.0
                ).astype(np.float32)
                masked.append((b, c, m))
    return L, cb, offs, masked


def _build(seq_lens):
    L, cb, offs, masked = _plan(seq_lens)
    C = int(offs[-1])
    nmask = len(masked)
    border = sorted(range(B), key=lambda b: cb[b])  # shortest first
    # order mask tiles by processing order so the early ones land first
    order = sorted(range(len(masked)), key=lambda i: (border.index(masked[i][0]), masked[i][1]))
    masked = [masked[i] for i in order]
    mask_np = np.concatenate([m for _, _, m in masked], axis=1).astype(
        ml_dtypes.bfloat16
    )  # [128, nm*256]; 0/-1e9 are bf16-exact
    mask_idx = {(b, c): i for i, (b, c, _) in enumerate(masked)}
    ident_np = np.eye(CHUNK, dtype=np.float32)
    identb_np = np.eye(CHUNK, dtype=ml_dtypes.bfloat16)
    ones_np = np.ones((CHUNK, 2), dtype=np.float32)

    nc = bacc.Bacc(
        "TRN2", target_bir_lowering=False, debug=False, num_devices=N_CORES
    )
    kt_d = nc.dram_tensor("kt", [D, C * CHUNK], F32R, kind="ExternalInput")
    v_d = nc.dram_tensor("v", [CHUNK, C * CHUNK], F32R, kind="ExternalInput")
    qt_d = nc.dram_tensor("qt", [D, B * QCOLS], F32R, kind="ExternalInput")
    out_d = nc.dram_tensor("out", [B, D, QCOLS], F32, kind="ExternalOutput")
    mask_d = nc.inline_tensor(mask_np, name="mask_const")
    identb_d = nc.inline_tensor(identb_np, name="identb_const")
    ident_d = nc.inline_tensor(ident_np, name="ident_const")
    ones_d = nc.inline_tensor(ones_np, name="ones_const")

    exp = mybir.ActivationFunctionType.Exp

    with tile.TileContext(nc) as tc:
        with (
            tc.tile_pool(name="sbin", bufs=1) as sbin,
            tc.tile_pool(name="sbu", bufs=6) as sbu,
            tc.tile_pool(name="sbe", bufs=3) as sbe,
            tc.tile_pool(name="ps_s", bufs=4, space="PSUM") as ps_s,
            tc.tile_pool(name="ps_o", bufs=1, space="PSUM") as ps_o,
            tc.tile_pool(name="ps_l", bufs=1, space="PSUM") as ps_l,
        ):
            # Critical-path DMAs first: K chunk 0 / first q half of the
            # first (longest) sequence, so the PE starts ~10us earlier.
            b0 = border[0]
            kt_t = [None] * B
            qt_t = [None] * B
            v_t = [None] * B
            w0 = cb[b0] * CHUNK
            kt_first = sbin.tile([D, w0], F32R, tag=f"kt{b0}")
            nc.sync.dma_start(
                kt_first[:, 0:CHUNK],
                kt_d.ap()[:, offs[b0] * CHUNK : offs[b0] * CHUNK + CHUNK],
            )
            qt_first = sbin.tile([D, QCOLS], F32R, tag=f"qt{b0}")
            nc.sync.dma_start(
                qt_first[:, 0:NHALF],
                qt_d.ap()[:, b0 * QCOLS : b0 * QCOLS + NHALF],
            )
            nc.sync.dma_start(
                qt_first[:, NHALF:QCOLS],
                qt_d.ap()[:, b0 * QCOLS + NHALF : (b0 + 1) * QCOLS],
            )
            v_first = sbin.tile([CHUNK, w0], F32R, tag=f"v{b0}")
            nc.sync.dma_start(
                v_first[:, 0 : 2 * CHUNK],
                v_d.ap()[:, offs[b0] * CHUNK : offs[b0] * CHUNK + 2 * CHUNK],
            )
            kcut = CHUNK
            vcut = 2 * CHUNK
            while kcut < w0 or vcut < w0:
                khi = min(kcut + 4 * CHUNK, w0)
                if khi > kcut:
                    nc.sync.dma_start(
                        kt_first[:, kcut:khi],
                        kt_d.ap()[
                            :, offs[b0] * CHUNK + kcut : offs[b0] * CHUNK + khi
                        ],
                    )
                    kcut = khi
                vhi = min(vcut + 4 * CHUNK, w0)
                if vhi > vcut:
                    nc.sync.dma_start(
                        v_first[:, vcut:vhi],
                        v_d.ap()[
                            :, offs[b0] * CHUNK + vcut : offs[b0] * CHUNK + vhi
                        ],
                    )
                    vcut = vhi
            kt_t[b0] = kt_first
            qt_t[b0] = qt_first

            identr = sbin.tile([CHUNK, CHUNK], mybir.dt.bfloat16, tag="identr")
            nc.sync.dma_start(identr[:], identb_d.ap())
            ones = sbin.tile([CHUNK, 2], F32R, tag="ones")
            nc.gpsimd.dma_start(ones[:], ones_d.ap())
            masks = sbin.tile([CHUNK, nmask * Q], mybir.dt.bfloat16, tag="masks")
            cut = Q * sum(
                1 for bb, _, _ in masked if cb[bb] <= cb[border[1]]
            )
            cut = max(Q, min(cut, nmask * Q))
            nc.sync.dma_start(masks[:, 0:cut], mask_d.ap()[:, 0:cut])
            if cut < nmask * Q:
                nc.sync.dma_start(
                    masks[:, cut : nmask * Q], mask_d.ap()[:, cut : nmask * Q]
                )

            for b in border:
                w = cb[b] * CHUNK
                head = min(2 * CHUNK, w)
                o0 = offs[b] * CHUNK
                if b == border[0]:
                    v_t[b] = v_first
                    continue
                vt = sbin.tile([CHUNK, w], F32R, tag=f"v{b}")
                if kt_t[b] is None:
                    kt = sbin.tile([D, w], F32R, tag=f"kt{b}")
                    nc.sync.dma_start(
                        kt[:, 0:head], kt_d.ap()[:, o0 : o0 + head]
                    )
                    qt = sbin.tile([D, QCOLS], F32R, tag=f"qt{b}")
                    nc.sync.dma_start(
                        qt[:], qt_d.ap()[:, b * QCOLS : (b + 1) * QCOLS]
                    )
                    nc.sync.dma_start(
                        vt[:, 0:head], v_d.ap()[:, o0 : o0 + head]
                    )
                    if head < w:
                        nc.sync.dma_start(
                            kt[:, head:w], kt_d.ap()[:, o0 + head : o0 + w]
                        )
                        nc.sync.dma_start(
                            vt[:, head:w], v_d.ap()[:, o0 + head : o0 + w]
                        )
                    kt_t[b] = kt
                    qt_t[b] = qt
                else:
                    nc.sync.dma_start(
                        vt[:, 0:head], v_d.ap()[:, o0 : o0 + head]
                    )
                    if head < w:
                        nc.sync.dma_start(
                            vt[:, head:w], v_d.ap()[:, o0 + head : o0 + w]
                        )
                v_t[b] = vt

            def half_state(b, c, n):
                # 'skip' = every q in the half is masked for this chunk;
                # 'mask' = the causal diagonal crosses this (chunk, half)
                lo = int(L[b]) - Q + n * CHUNK
                if c * CHUNK > lo + CHUNK - 1:
                    return "skip"
                if c * CHUNK + CHUNK - 1 > lo:
                    return "mask"
                return "clear"

            def emit_score(b, c):
                mi = mask_idx.get((b, c))
                u_h = []
                for n in range(2):
                    st = half_state(b, c, n)
                    if st == "skip":
                        u_h.append(None)
                        continue
                    s_ps = ps_s.tile([CHUNK, NHALF], F32, tag="s")
                    nc.tensor.matmul(
                        s_ps[:],
                        kt_t[b][:, c * CHUNK : (c + 1) * CHUNK],
                        qt_t[b][:, n * NHALF : (n + 1) * NHALF],
                        start=True,
                        stop=st == "clear",
                    )
                    if st == "mask":
                        mb = (
                            masks[
                                :,
                                mi * Q + n * CHUNK : mi * Q + (n + 1) * CHUNK,
                            ]
                            .unsqueeze(2)
                            .broadcast_to([CHUNK, CHUNK, G])
                        )
                        nc.tensor.matmul(
                            s_ps[:], identr[:], mb, start=False, stop=True
                        )
                    u = sbu.tile([CHUNK, NHALF], F32R, tag="u")
                    nc.scalar.activation(u[:], s_ps[:], exp, scale=SCALE)
                    u_h.append(u)
                return u_h

            def emit_consume(b, c, u_h, o_ps, l_ps, last_n):
                for n in range(2):
                    if u_h[n] is None:
                        continue
                    nc.tensor.matmul(
                        l_ps[:, n * NHALF : (n + 1) * NHALF],
                        ones[:, 0:2],
                        u_h[n][:],
                        start=c == 0,
                        stop=c == last_n[n],
                    )
                for n in range(2):
                    if u_h[n] is None:
                        continue
                    nc.tensor.matmul(
                        o_ps[:, n * NHALF : (n + 1) * NHALF],
                        v_t[b][:, c * CHUNK : (c + 1) * CHUNK],
                        u_h[n][:],
                        start=c == 0,
                        stop=c == last_n[n],
                    )

            u0_next = None
            for bi, b in enumerate(border):
                terminal = bi == len(border) - 1
                nchunks = cb[b]
                # last contributing chunk per half (later ones are skipped)
                last_n = [
                    min(nchunks - 1, (int(L[b]) - Q + n * CHUNK + CHUNK - 1) // CHUNK)
                    for n in range(2)
                ]
                o_ps = ps_o.tile([D, QCOLS], F32, tag="o")
                l_ps = ps_l.tile([2, QCOLS], F32, tag="l")
                for c in range(nchunks):
                    if c == 0 and u0_next is not None:
                        u_h = u0_next
                        u0_next = None
                    else:
                        u_h = emit_score(b, c)
                    emit_consume(b, c, u_h, o_ps, l_ps, last_n)

                # epilogue: rl = 1/l broadcast down partitions, one multiply.
                # o is copied out of PSUM immediately so the next sequence's
                # PV accumulation can claim the banks.
                l_sb = sbe.tile([1, QCOLS], F32, tag="lsb")
                if terminal:
                    # tail chain: l-copy on the (now idle) ScalarE, and read
                    # O straight from PSUM -- no next sequence needs the banks
                    nc.scalar.copy(l_sb[:], l_ps[0:1, :])
                    osrc = o_ps
                else:
                    nc.vector.tensor_copy(l_sb[:], l_ps[0:1, :])
                    ocp = sbe.tile([D, QCOLS], F32, tag="ocp")
                    nc.vector.tensor_copy(ocp[:], o_ps[:])
                    osrc = ocp
                rl_row = sbe.tile([1, QCOLS], F32, tag="rlrow")
                rlb = sbe.tile([D, QCOLS], F32, tag="rlb")
                out_sb = sbe.tile([D, QCOLS], F32, tag="osb")
                for n in range(2):
                    half = slice(n * NHALF, (n + 1) * NHALF)
                    nc.vector.reciprocal_approx_fast(
                        rl_row[:, half], l_sb[:, half]
                    )
                    nc.gpsimd.partition_broadcast(
                        rlb[:, half], rl_row[:, half]
                    )
                    nc.vector.tensor_mul(
                        out_sb[:, half], osrc[:, half], rlb[:, half]
                    )
                    nc.sync.dma_start(
                        out_d.ap()[b][:, half], out_sb[:, half]
                    )

    nc.compile()
    return nc, L, cb, offs


def _pack_inputs(query, k_cache, v_cache, block_tables, L, cb, offs):
    """Gather the paged cache and pack per-core shards in device layouts."""
    C = int(offs[-1])
    k_lin = k_cache[block_tables].reshape(B, KV, KVH, D)
    v_lin = v_cache[block_tables].reshape(B, KV, KVH, D)
    kt_all = np.zeros((KVH, D, C * CHUNK), dtype=np.float32)
    v_all = np.zeros((KVH, CHUNK, C * CHUNK), dtype=np.float32)
    for b in range(B):
        Lb, w = int(L[b]), cb[b] * CHUNK
        kk = np.zeros((w, KVH, D), dtype=np.float32)
        kk[:Lb] = k_lin[b, :Lb]
        # [w, KVH, D] -> [KVH, D, w]
        kt_all[:, :, offs[b] * CHUNK : offs[b] * CHUNK + w] = kk.transpose(
            1, 2, 0
        )
        vv = np.zeros((w, KVH, D), dtype=np.float32)
        vv[:Lb] = v_lin[b, :Lb]
        # [cb, 128, KVH, D] -> [KVH, 128, cb, D] -> [KVH, 128, w]
        v_all[:, :, offs[b] * CHUNK : offs[b] * CHUNK + w] = (
            vv.reshape(cb[b], CHUNK, KVH, D)
            .transpose(2, 1, 0, 3)
            .reshape(KVH, CHUNK, w)
        )
    # query [B,Q,H,D] -> [KVH, D, B, Q, G] (t-major, g inner)
    qt_all = (
        query.transpose(2, 3, 0, 1)
        .reshape(KVH, G, D, B, Q)
        .transpose(0, 2, 3, 4, 1)
        .reshape(KVH, D, B * QCOLS)
    )
    qt_all = np.ascontiguousarray(qt_all, dtype=np.float32)
    return [
        {
            "kt": np.ascontiguousarray(kt_all[h]),
            "v": np.ascontiguousarray(v_all[h]),
            "qt": qt_all[h],
        }
        for h in range(KVH)
    ]


def _unpack_outputs(results):
    """[B,D,QCOLS] per core (O^T, q=(g,t) on cols) -> [B*Q, H*D]."""
    out = np.empty((B * Q, H * D), dtype=np.float32)
    for h, res in enumerate(results):
        o = res["out"].reshape(B, D, Q, G)  # [b, d, t, g]
        o = o.transpose(0, 2, 3, 1).reshape(B * Q, G * D)
        out[:, h * G * D : (h + 1) * G * D] = o
    return out


def kernel(query, k_cache, v_cache, block_tables, seq_lens):
    query = np.asarray(query, dtype=np.float32)
    k_cache = np.asarray(k_cache, dtype=np.float32)
    v_cache = np.asarray(v_cache, dtype=np.float32)
    block_tables = np.asarray(block_tables, dtype=np.int64)
    nc, L, cb, offs = _build(np.asarray(seq_lens))
    in_maps = _pack_inputs(query, k_cache, v_cache, block_tables, L, cb, offs)
    res = run_bass_kernel_spmd(nc, in_maps, core_ids=list(range(N_CORES)))
    return _unpack_outputs(res.results)



# revision 11
# speedup vs baseline: 1.0759x; 1.0759x over previous
"""Paged GQA chunked-prefill attention for 8 Trainium2 NeuronCores.

Problem (hardcoded): B=4 seqs x Q=256 new tokens, H=32 query heads, KVH=8 kv
heads (GQA group G=4), D=128 head dim, paged KV cache of 512 blocks x 16
tokens, per-seq lengths in seq_lens (clamped to >= Q), causal masking.

Sharding: tensor-parallel over heads. Core h gets kv head h and query heads
h*4..h*4+3; block_tables/seq_lens are resolved host-side while packing the
shards; the output is all-gathered host-side over the hidden dim.

Per-core device kernel, all matmul operands fp16 (full PE rate, fp32 PSUM
accumulation). For seq b, kv chunk c of 128 positions, q = (t,g) -> 1024
columns in two 512-column halves n:
  S^T[kv,q] = K_c^T q                (fp16 matmul into PSUM)
  S^T += causal mask                 (identity-lhsT matmul, same bank group)
  U = exp(SCALE * S^T)               (one ScalarE pass over both halves)
  l[q]  += ones^T @ U                (4-way col-tiled thin matmuls: quarter j
                                      lands on PSUM partition 32j, ~4x faster
                                      than a full-width pass)
  O^T[d,q] += V_c^T @ U              (PSUM accumulation over chunks)
Scores/exp are emitted one chunk ahead of l/PV consumption so the PE never
stalls behind the ScalarE exp. Epilogue: copy l out of PSUM, fast reciprocal,
gpsimd partition-broadcast per quarter, one fused multiply to fp16, DMA out.
"""
import math

import numpy as np

import concourse.mybir as mybir
import concourse.tile as tile
from concourse import bacc
from concourse.bass_utils import run_bass_kernel_spmd

B, Q, H, D = 4, 256, 32, 128
KVH = 8
G = H // KVH
BLOCK = 16
NB = 128
KV = NB * BLOCK
NUM_BLOCKS = B * NB
SCALE = 1.0 / math.sqrt(D)
N_CORES = 8
CHUNK = 128
QCOLS = G * Q  # 1024 q columns per sequence per core
NHALF = 512

F32 = mybir.dt.float32
F16 = mybir.dt.float16
NEG = -60000.0  # exactly representable in fp16; SCALE*NEG ~ -5300 -> exp = 0


def _plan(seq_lens):
    """Chunk counts, processing order, and boundary-chunk mask tiles."""
    L = np.maximum(np.asarray(seq_lens, dtype=np.int64), Q)
    cb = [int((int(x) + CHUNK - 1) // CHUNK) for x in L]
    offs = np.concatenate([[0], np.cumsum(cb)]).astype(int)
    border = sorted(range(B), key=lambda b: cb[b])  # shortest first
    masked = {}
    p = np.arange(CHUNK)
    t = np.arange(CHUNK)
    for b in range(B):
        for c in range(cb[b]):
            for n in range(2):
                lo = int(L[b]) - Q + n * CHUNK  # kv pos of this half's t=0
                if c * CHUNK > lo + CHUNK - 1:
                    continue  # fully masked half
                if c * CHUNK + CHUNK - 1 > lo:
                    kvpos = c * CHUNK + p
                    m = np.where(
                        kvpos[:, None] > lo + t[None, :], NEG, 0.0
                    ).astype(np.float16)
                    masked[(b, c, n)] = m
    order = sorted(
        masked.keys(), key=lambda k: (border.index(k[0]), k[1], k[2])
    )
    return L, cb, offs, border, masked, order


def _build(seq_lens):
    L, cb, offs, border, masked, order = _plan(seq_lens)
    C = int(offs[-1])
    nm = len(order)
    mask_np = np.concatenate([masked[k] for k in order], axis=1)  # [128, nm*128]
    midx = {k: i for i, k in enumerate(order)}
    ident_np = np.eye(CHUNK, dtype=np.float16)
    ones_np = np.ones((CHUNK, 1), dtype=np.float16)

    nc = bacc.Bacc(
        "TRN2", target_bir_lowering=False, debug=False, num_devices=N_CORES
    )
    kt_d = nc.dram_tensor("kt", [D, C * CHUNK], F16, kind="ExternalInput")
    v_d = nc.dram_tensor("v", [CHUNK, C * CHUNK], F16, kind="ExternalInput")
    qt_d = nc.dram_tensor("qt", [D, B * QCOLS], F16, kind="ExternalInput")
    out_d = nc.dram_tensor("out", [B, D, QCOLS], F16, kind="ExternalOutput")
    mask_d = nc.inline_tensor(mask_np, name="mask_const")
    ident_d = nc.inline_tensor(ident_np, name="ident_const")
    ones_d = nc.inline_tensor(ones_np, name="ones_const")

    exp = mybir.ActivationFunctionType.Exp

    def half_lo(b, n):
        return int(L[b]) - Q + n * CHUNK

    def half_state(b, c, n):
        if c * CHUNK > half_lo(b, n) + CHUNK - 1:
            return "skip"
        if (b, c, n) in midx:
            return "mask"
        return "clear"

    def last_chunk(b, n):
        return min(cb[b] - 1, (half_lo(b, n) + CHUNK - 1) // CHUNK)

    with tile.TileContext(nc) as tc:
        with (
            tc.tile_pool(name="sbin", bufs=1) as sbin,
            tc.tile_pool(name="sbu", bufs=4) as sbu,
            tc.tile_pool(name="sbe", bufs=2) as sbe,
            tc.tile_pool(name="ps_s", bufs=2, space="PSUM") as ps_s,
            tc.tile_pool(name="ps_o", bufs=1, space="PSUM") as ps_o,
            tc.tile_pool(name="ps_l", bufs=1, space="PSUM") as ps_l,
            tc.tile_pool(name="ps_w", bufs=1, space="PSUM") as ps_w,
        ):
            # Tiny consts first so the ACT table preload + PE warmup can
            # start while the bulk input DMAs stream in.
            ones_t = sbin.tile([CHUNK, 1], F16, tag="ones")
            nc.sync.dma_start(ones_t[:], ones_d.ap())
            identr = sbin.tile([CHUNK, CHUNK], F16, tag="identr")
            nc.sync.dma_start(identr[:], ident_d.ap())
            dummy = sbe.tile([CHUNK, 1], F32, tag="dummy")
            nc.scalar.activation(dummy[:], ones_t[:], exp)  # table load now
            warm_ps = ps_w.tile([CHUNK, NHALF], F32, tag="warm")
            for _ in range(12):  # keep the PE HAM window busy from t=0
                nc.tensor.matmul(
                    warm_ps[:, 0:CHUNK], identr[:], identr[:],
                    start=True, stop=True,
                )

            # Input DMAs in processing order; first sequence split fine.
            b0 = border[0]
            kt_t = [None] * B
            v_t = [None] * B
            qt_t = [None] * B
            w0 = cb[b0] * CHUNK
            qt0 = sbin.tile([D, QCOLS], F16, tag=f"qt{b0}")
            nc.sync.dma_start(
                qt0[:, 0:NHALF], qt_d.ap()[:, b0 * QCOLS : b0 * QCOLS + NHALF]
            )
            kt0 = sbin.tile([D, w0], F16, tag=f"kt{b0}")
            nc.sync.dma_start(
                kt0[:], kt_d.ap()[:, offs[b0] * CHUNK : offs[b0] * CHUNK + w0]
            )
            v0 = sbin.tile([CHUNK, w0], F16, tag=f"v{b0}")
            nc.sync.dma_start(
                v0[:], v_d.ap()[:, offs[b0] * CHUNK : offs[b0] * CHUNK + w0]
            )
            nc.sync.dma_start(
                qt0[:, NHALF:QCOLS],
                qt_d.ap()[:, b0 * QCOLS + NHALF : (b0 + 1) * QCOLS],
            )
            kt_t[b0], v_t[b0], qt_t[b0] = kt0, v0, qt0

            masks = sbin.tile([CHUNK, max(nm, 1) * CHUNK], F16, tag="masks")
            cut = CHUNK * sum(
                1 for (bb, _, _) in order if border.index(bb) <= 1
            )
            cut = max(CHUNK, min(cut, nm * CHUNK))
            if nm:
                nc.sync.dma_start(masks[:, 0:cut], mask_d.ap()[:, 0:cut])

            for b in border[1:]:
                w = cb[b] * CHUNK
                o0 = offs[b] * CHUNK
                qt = sbin.tile([D, QCOLS], F16, tag=f"qt{b}")
                nc.sync.dma_start(qt[:], qt_d.ap()[:, b * QCOLS : (b + 1) * QCOLS])
                kt = sbin.tile([D, w], F16, tag=f"kt{b}")
                vt = sbin.tile([CHUNK, w], F16, tag=f"v{b}")
                head = min(4 * CHUNK, w)
                nc.sync.dma_start(kt[:, 0:head], kt_d.ap()[:, o0 : o0 + head])
                nc.sync.dma_start(vt[:, 0:head], v_d.ap()[:, o0 : o0 + head])
                if head < w:
                    nc.sync.dma_start(
                        kt[:, head:w], kt_d.ap()[:, o0 + head : o0 + w]
                    )
                    nc.sync.dma_start(
                        vt[:, head:w], v_d.ap()[:, o0 + head : o0 + w]
                    )
                kt_t[b], v_t[b], qt_t[b] = kt, vt, qt
            if nm and cut < nm * CHUNK:
                nc.sync.dma_start(
                    masks[:, cut : nm * CHUNK], mask_d.ap()[:, cut : nm * CHUNK]
                )

            # l rows: quarter j -> partition 32j, cols (n, k). One accumulation
            # group per row: a col-tiled start=True matmul clears has_written
            # for its whole partition row, so two groups sharing a row would
            # wipe each other's first chunk.
            l_ps = ps_l.tile([CHUNK, 2 * CHUNK], F32, tag="l")
            o_ps = ps_o.tile([D, QCOLS], F32, tag="o")

            def emit_score(b, c):
                states = [half_state(b, c, n) for n in range(2)]
                s_ps = ps_s.tile([CHUNK, QCOLS], F32, tag="s")
                for n in range(2):
                    if states[n] == "skip":
                        continue
                    nc.tensor.matmul(
                        s_ps[:, n * NHALF : (n + 1) * NHALF],
                        kt_t[b][:, c * CHUNK : (c + 1) * CHUNK],
                        qt_t[b][:, n * NHALF : (n + 1) * NHALF],
                        start=True,
                        stop=states[n] == "clear",
                    )
                    if states[n] == "mask":
                        mi = midx[(b, c, n)]
                        mb = (
                            masks[:, mi * CHUNK : (mi + 1) * CHUNK]
                            .unsqueeze(2)
                            .broadcast_to([CHUNK, CHUNK, G])
                        )
                        nc.tensor.matmul(
                            s_ps[:, n * NHALF : (n + 1) * NHALF],
                            identr[:],
                            mb,
                            start=False,
                            stop=True,
                        )
                col0 = 0 if states[0] != "skip" else NHALF
                u = sbu.tile([CHUNK, QCOLS], F16, tag="u")
                nc.scalar.activation(
                    u[:, col0:QCOLS], s_ps[:, col0:QCOLS], exp, scale=SCALE
                )
                return u, states

            def emit_consume(b, c, u, states):
                last = [last_chunk(b, n) for n in range(2)]
                u_nk = u[:].rearrange("p (n j k) -> p j n k", n=2, j=4, k=CHUNK)
                for j in range(4):
                    if states[0] != "skip":  # both halves in one 256-col MM
                        nc.tensor.matmul(
                            l_ps[32 * j : 32 * j + 1, 0 : 2 * CHUNK],
                            ones_t[:, 0:1],
                            u_nk[:, j : j + 1],
                            start=c == 0,
                            stop=c == last[1],
                            skip_group_check=True,
                            tile_position=(0, 32 * j),
                        )
                    else:  # half 0 done for this seq: accumulate half 1 only
                        nc.tensor.matmul(
                            l_ps[32 * j : 32 * j + 1, CHUNK : 2 * CHUNK],
                            ones_t[:, 0:1],
                            u[:, NHALF + j * CHUNK : NHALF + (j + 1) * CHUNK],
                            start=c == 0,
                            stop=c == last[1],
                            skip_group_check=True,
                            tile_position=(0, 32 * j),
                        )
                for n in range(2):
                    if states[n] == "skip":
                        continue
                    nc.tensor.matmul(
                        o_ps[:, n * NHALF : (n + 1) * NHALF],
                        v_t[b][:, c * CHUNK : (c + 1) * CHUNK],
                        u[:, n * NHALF : (n + 1) * NHALF],
                        start=c == 0,
                        stop=c == last[n],
                    )

            def emit_epilogue(b, terminal):
                l_sbf = sbe.tile([CHUNK, 2 * CHUNK], F32, tag="lsb")
                nc.vector.tensor_copy(l_sbf[:], l_ps[:])
                rl = sbe.tile([CHUNK, 2 * CHUNK], F32, tag="rl")
                nc.vector.reciprocal_approx_fast(rl[:], l_sbf[:])
                # Gather rows {32j} to partition 0 (broadcast ucode wants
                # partition 0), reordering cols (j, n, k) -> q = n*512+j*128+k.
                lrow = sbe.tile([1, QCOLS], F32, tag="lrow")
                for n in range(2):
                    nc.sync.dma_start(
                        lrow[0:1, n * NHALF : (n + 1) * NHALF].rearrange(
                            "p (j k) -> p j k", j=4, k=CHUNK
                        ),
                        rl[0:CHUNK:32, n * CHUNK : (n + 1) * CHUNK],
                    )
                rlb = sbe.tile([CHUNK, QCOLS], F32, tag="rlb")
                for n in range(2):
                    nc.gpsimd.partition_broadcast(
                        rlb[:, n * NHALF : (n + 1) * NHALF],
                        lrow[0:1, n * NHALF : (n + 1) * NHALF],
                    )
                if terminal:
                    osrc = o_ps
                else:
                    ocp = sbe.tile([D, QCOLS], F32, tag="ocp")
                    nc.vector.tensor_copy(ocp[:], o_ps[:])
                    osrc = ocp
                out_sb = sbe.tile([D, QCOLS], F16, tag="osb")
                for n in range(2):
                    half = slice(n * NHALF, (n + 1) * NHALF)
                    nc.vector.tensor_mul(
                        out_sb[:, half], osrc[:, half], rlb[:, half]
                    )
                    nc.sync.dma_start(out_d.ap()[b][:, half], out_sb[:, half])

            flat = [(b, c) for b in border for c in range(cb[b])]
            pend = None
            for b, c in flat:
                u, states = emit_score(b, c)
                if pend is not None:
                    pb, pc, pu, pst = pend
                    emit_consume(pb, pc, pu, pst)
                    if pc == cb[pb] - 1:
                        emit_epilogue(pb, terminal=False)
                pend = (b, c, u, states)
            pb, pc, pu, pst = pend
            emit_consume(pb, pc, pu, pst)
            emit_epilogue(pb, terminal=True)

    nc.compile()
    return nc, L, cb, offs


def _pack_inputs(query, k_cache, v_cache, block_tables, L, cb, offs):
    """Gather the paged cache and pack per-core fp16 shards in device layouts."""
    C = int(offs[-1])
    k_lin = k_cache[block_tables].reshape(B, KV, KVH, D)
    v_lin = v_cache[block_tables].reshape(B, KV, KVH, D)
    kt_all = np.zeros((KVH, D, C * CHUNK), dtype=np.float32)
    v_all = np.zeros((KVH, CHUNK, C * CHUNK), dtype=np.float32)
    for b in range(B):
        Lb, w = int(L[b]), cb[b] * CHUNK
        kk = np.zeros((w, KVH, D), dtype=np.float32)
        kk[:Lb] = k_lin[b, :Lb]
        kt_all[:, :, offs[b] * CHUNK : offs[b] * CHUNK + w] = kk.transpose(
            1, 2, 0
        )
        vv = np.zeros((w, KVH, D), dtype=np.float32)
        vv[:Lb] = v_lin[b, :Lb]
        # [cb, 128, KVH, D] -> [KVH, 128, cb*D]
        v_all[:, :, offs[b] * CHUNK : offs[b] * CHUNK + w] = (
            vv.reshape(cb[b], CHUNK, KVH, D)
            .transpose(2, 1, 0, 3)
            .reshape(KVH, CHUNK, w)
        )
    # query [B,Q,H,D] -> [KVH, D, B, Q, G] (t-major, g inner)
    qt_all = (
        query.transpose(2, 3, 0, 1)
        .reshape(KVH, G, D, B, Q)
        .transpose(0, 2, 3, 4, 1)
        .reshape(KVH, D, B * QCOLS)
    )
    kt_all = kt_all.astype(np.float16)
    v_all = v_all.astype(np.float16)
    qt_all = np.ascontiguousarray(qt_all).astype(np.float16)
    return [
        {
            "kt": np.ascontiguousarray(kt_all[h]),
            "v": np.ascontiguousarray(v_all[h]),
            "qt": qt_all[h],
        }
        for h in range(KVH)
    ]


def _unpack_outputs(results):
    """[B,D,QCOLS] fp16 per core (O^T, q=(t,g) on cols) -> [B*Q, H*D] f32."""
    out = np.empty((B * Q, H * D), dtype=np.float32)
    for h, res in enumerate(results):
        o = res["out"].astype(np.float32).reshape(B, D, Q, G)  # [b, d, t, g]
        o = o.transpose(0, 2, 3, 1).reshape(B * Q, G * D)
        out[:, h * G * D : (h + 1) * G * D] = o
    return out


def kernel(query, k_cache, v_cache, block_tables, seq_lens):
    query = np.asarray(query, dtype=np.float32)
    k_cache = np.asarray(k_cache, dtype=np.float32)
    v_cache = np.asarray(v_cache, dtype=np.float32)
    block_tables = np.asarray(block_tables, dtype=np.int64)
    nc, L, cb, offs = _build(np.asarray(seq_lens))
    in_maps = _pack_inputs(query, k_cache, v_cache, block_tables, L, cb, offs)
    res = run_bass_kernel_spmd(nc, in_maps, core_ids=list(range(N_CORES)))
    return _unpack_outputs(res.results)


# revision 14
# speedup vs baseline: 1.1310x; 1.0512x over previous
"""Paged GQA chunked-prefill attention for 8 Trainium2 NeuronCores.

Problem (hardcoded): B=4 seqs x Q=256 new tokens, H=32 query heads, KVH=8 kv
heads (GQA group G=4), D=128 head dim, paged KV cache of 512 blocks x 16
tokens, per-seq lengths in seq_lens (clamped to >= Q), causal masking.

Sharding: tensor-parallel over heads. Core h gets kv head h and query heads
h*4..h*4+3; block_tables/seq_lens are resolved host-side while packing the
shards; the output is all-gathered host-side over the hidden dim.

Per-core device kernel, all matmul operands fp16 (full PE rate, fp32 PSUM
accumulation). For seq b, kv chunk c of 128 positions, q = (t,g) -> 1024
columns in two 512-column halves n; fully-masked column prefixes of boundary
chunks are clipped out of every stage:
  S^T[kv,q] = K_c^T q                 (fp16 matmul into PSUM)
  S^T += causal mask                  (identity-lhsT matmul, boundary band)
  U = exp(SCALE * S^T)                (one ScalarE pass over the active cols)
  lb[128,q] += ones128^T @ U          (all-ones lhsT: the denominator lands
                                       broadcast across all 128 partitions,
                                       so the epilogue needs no cross-
                                       partition moves at all)
  O^T[d,q] += V_c^T @ U               (PSUM accumulation over chunks)
Scores/exp are emitted one chunk ahead of l/PV consumption so the PE never
stalls behind the ScalarE exp. Epilogue per half: rl = 1/lb (fast approx
reciprocal straight out of PSUM), out = O^T * rl -> fp16, DMA out. The PE is
kept warm from t=0 with matmuls on a memset tile while input DMAs stream.
"""
import math

import numpy as np

import concourse.mybir as mybir
import concourse.tile as tile
from concourse import bacc
from concourse.bass_utils import run_bass_kernel_spmd

B, Q, H, D = 4, 256, 32, 128
KVH = 8
G = H // KVH
BLOCK = 16
NB = 128
KV = NB * BLOCK
NUM_BLOCKS = B * NB
SCALE = 1.0 / math.sqrt(D)
N_CORES = 8
CHUNK = 128
QCOLS = G * Q  # 1024 q columns per sequence per core
NHALF = 512

F32 = mybir.dt.float32
F16 = mybir.dt.float16
NEG = -60000.0  # exactly representable in fp16; SCALE*NEG ~ -5300 -> exp = 0


def _plan(seq_lens):
    """Chunk counts, processing order, and tight boundary mask tiles."""
    L = np.maximum(np.asarray(seq_lens, dtype=np.int64), Q)
    cb = [int((int(x) + CHUNK - 1) // CHUNK) for x in L]
    offs = np.concatenate([[0], np.cumsum(cb)]).astype(int)
    border = sorted(range(B), key=lambda b: cb[b])  # shortest first
    # (b, c, n) -> (tmin, tup, mask[128, tup-tmin]); cols t < tmin are fully
    # masked (clipped everywhere), t >= tup fully visible.
    masked = {}
    p = np.arange(CHUNK)
    for b in range(B):
        for c in range(cb[b]):
            for n in range(2):
                lo = int(L[b]) - Q + n * CHUNK  # kv pos of this half's t=0
                if c * CHUNK > lo + CHUNK - 1:
                    continue  # fully masked half
                if c * CHUNK + CHUNK - 1 > lo:
                    tmin = max(0, min(CHUNK, c * CHUNK - lo))
                    tup = max(0, min(CHUNK, c * CHUNK + CHUNK - lo))
                    t = np.arange(tmin, tup)
                    kvpos = c * CHUNK + p
                    m = np.where(
                        kvpos[:, None] > lo + t[None, :], NEG, 0.0
                    ).astype(np.float16)
                    masked[(b, c, n)] = (tmin, tup, m)
    order = sorted(
        masked.keys(), key=lambda k: (border.index(k[0]), k[1], k[2])
    )
    return L, cb, offs, border, masked, order


def _build(seq_lens):
    L, cb, offs, border, masked, order = _plan(seq_lens)
    C = int(offs[-1])
    mask_np = np.concatenate(
        [masked[k][2] for k in order] or [np.zeros((CHUNK, 1), np.float16)],
        axis=1,
    )
    mcols = mask_np.shape[1]
    moff = {}
    acc = 0
    for k in order:
        moff[k] = acc
        acc += masked[k][2].shape[1]
    ident_np = np.eye(CHUNK, dtype=np.float16)
    ones_np = np.ones((CHUNK, CHUNK), dtype=np.float16)

    nc = bacc.Bacc(
        "TRN2", target_bir_lowering=False, debug=False, num_devices=N_CORES
    )
    kt_d = nc.dram_tensor("kt", [D, C * CHUNK], F16, kind="ExternalInput")
    v_d = nc.dram_tensor("v", [CHUNK, C * CHUNK], F16, kind="ExternalInput")
    qt_d = nc.dram_tensor("qt", [D, B * QCOLS], F16, kind="ExternalInput")
    out_d = nc.dram_tensor("out", [B, D, QCOLS], F16, kind="ExternalOutput")
    mask_d = nc.inline_tensor(mask_np, name="mask_const")
    ident_d = nc.inline_tensor(ident_np, name="ident_const")
    ones_d = nc.inline_tensor(ones_np, name="ones_const")

    exp = mybir.ActivationFunctionType.Exp

    def half_lo(b, n):
        return int(L[b]) - Q + n * CHUNK

    def half_state(b, c, n):
        if c * CHUNK > half_lo(b, n) + CHUNK - 1:
            return "skip"
        if (b, c, n) in masked:
            return "mask"
        return "clear"

    def last_chunk(b, n):
        return min(cb[b] - 1, (half_lo(b, n) + CHUNK - 1) // CHUNK)

    def clip_a(b, c, n):
        """First active column (of 512) for this chunk-half."""
        if (b, c, n) in masked:
            return G * masked[(b, c, n)][0]
        return 0

    with tile.TileContext(nc) as tc:
        with (
            tc.tile_pool(name="sbin", bufs=1) as sbin,
            tc.tile_pool(name="sbu", bufs=4) as sbu,
            tc.tile_pool(name="sbe", bufs=2) as sbe,
            tc.tile_pool(name="ps_s", bufs=2, space="PSUM") as ps_s,
            tc.tile_pool(name="ps_o", bufs=1, space="PSUM") as ps_o,
            tc.tile_pool(name="ps_l", bufs=1, space="PSUM") as ps_l,
        ):
            # Warm the PE + load the exp table before any DMA lands: matmuls
            # on a memset tile keep the HAM activity window busy so the clock
            # is at 2.4 GHz when real data arrives.
            warm_w = sbin.tile([CHUNK, CHUNK], F16, tag="warmw")
            nc.vector.memset(warm_w[:], 0.0)
            dummy = sbe.tile([CHUNK, 1], F32, tag="dummy")
            nc.scalar.activation(dummy[:], warm_w[:, 0:1], exp)
            # l broadcast accumulators double as the warmup target; the first
            # real l matmul (start=True) resets them.
            l_bc = ps_l.tile([CHUNK, QCOLS], F32, tag="lbc")
            for _ in range(26):
                nc.tensor.matmul(
                    l_bc[:, 0:CHUNK], warm_w[:], warm_w[:],
                    start=True, stop=True, skip_group_check=True,
                )

            # Input DMAs in processing order; first sequence split fine.
            b0 = border[0]
            kt_t = [None] * B
            v_t = [None] * B
            qt_t = [None] * B
            w0 = cb[b0] * CHUNK
            qt0 = sbin.tile([D, QCOLS], F16, tag=f"qt{b0}")
            nc.sync.dma_start(
                qt0[:, 0:NHALF], qt_d.ap()[:, b0 * QCOLS : b0 * QCOLS + NHALF]
            )
            kt0 = sbin.tile([D, w0], F16, tag=f"kt{b0}")
            nc.sync.dma_start(
                kt0[:], kt_d.ap()[:, offs[b0] * CHUNK : offs[b0] * CHUNK + w0]
            )
            v0 = sbin.tile([CHUNK, w0], F16, tag=f"v{b0}")
            nc.sync.dma_start(
                v0[:], v_d.ap()[:, offs[b0] * CHUNK : offs[b0] * CHUNK + w0]
            )
            nc.sync.dma_start(
                qt0[:, NHALF:QCOLS],
                qt_d.ap()[:, b0 * QCOLS + NHALF : (b0 + 1) * QCOLS],
            )
            identr = sbin.tile([CHUNK, CHUNK], F16, tag="identr")
            nc.sync.dma_start(identr[:], ident_d.ap())
            ones_t = sbin.tile([CHUNK, CHUNK], F16, tag="ones")
            nc.sync.dma_start(ones_t[:], ones_d.ap())
            kt_t[b0], v_t[b0], qt_t[b0] = kt0, v0, qt0

            masks = sbin.tile([CHUNK, mcols], F16, tag="masks")
            cut = sum(
                masked[k][2].shape[1]
                for k in order
                if border.index(k[0]) <= 1
            )
            cut = max(1, min(cut, mcols))
            nc.sync.dma_start(masks[:, 0:cut], mask_d.ap()[:, 0:cut])

            for b in border[1:]:
                w = cb[b] * CHUNK
                o0 = offs[b] * CHUNK
                qt = sbin.tile([D, QCOLS], F16, tag=f"qt{b}")
                nc.sync.dma_start(qt[:], qt_d.ap()[:, b * QCOLS : (b + 1) * QCOLS])
                kt = sbin.tile([D, w], F16, tag=f"kt{b}")
                vt = sbin.tile([CHUNK, w], F16, tag=f"v{b}")
                head = min(4 * CHUNK, w)
                nc.sync.dma_start(kt[:, 0:head], kt_d.ap()[:, o0 : o0 + head])
                nc.sync.dma_start(vt[:, 0:head], v_d.ap()[:, o0 : o0 + head])
                if head < w:
                    nc.sync.dma_start(
                        kt[:, head:w], kt_d.ap()[:, o0 + head : o0 + w]
                    )
                    nc.sync.dma_start(
                        vt[:, head:w], v_d.ap()[:, o0 + head : o0 + w]
                    )
                kt_t[b], v_t[b], qt_t[b] = kt, vt, qt
            if cut < mcols:
                nc.sync.dma_start(
                    masks[:, cut:mcols], mask_d.ap()[:, cut:mcols]
                )

            o_ps = ps_o.tile([D, QCOLS], F32, tag="o")

            def emit_score(b, c):
                states = [half_state(b, c, n) for n in range(2)]
                s_ps = ps_s.tile([CHUNK, QCOLS], F32, tag="s")
                for n in range(2):
                    if states[n] == "skip":
                        continue
                    h0 = n * NHALF
                    if states[n] == "clear":
                        nc.tensor.matmul(
                            s_ps[:, h0 : h0 + NHALF],
                            kt_t[b][:, c * CHUNK : (c + 1) * CHUNK],
                            qt_t[b][:, h0 : h0 + NHALF],
                            start=True,
                            stop=True,
                        )
                        continue
                    tmin, tup, _ = masked[(b, c, n)]
                    a, u_c = G * tmin, G * tup
                    nc.tensor.matmul(
                        s_ps[:, h0 + a : h0 + u_c],
                        kt_t[b][:, c * CHUNK : (c + 1) * CHUNK],
                        qt_t[b][:, h0 + a : h0 + u_c],
                        start=True,
                        stop=False,
                    )
                    mo = moff[(b, c, n)]
                    mb = (
                        masks[:, mo : mo + (tup - tmin)]
                        .unsqueeze(2)
                        .broadcast_to([CHUNK, tup - tmin, G])
                    )
                    nc.tensor.matmul(
                        s_ps[:, h0 + a : h0 + u_c],
                        identr[:],
                        mb,
                        start=False,
                        stop=True,
                    )
                    if u_c < NHALF:
                        nc.tensor.matmul(
                            s_ps[:, h0 + u_c : h0 + NHALF],
                            kt_t[b][:, c * CHUNK : (c + 1) * CHUNK],
                            qt_t[b][:, h0 + u_c : h0 + NHALF],
                            start=True,
                            stop=True,
                        )
                act0 = (
                    clip_a(b, c, 0)
                    if states[0] != "skip"
                    else NHALF + clip_a(b, c, 1)
                )
                u = sbu.tile([CHUNK, QCOLS], F16, tag="u")
                nc.scalar.activation(
                    u[:, act0:QCOLS], s_ps[:, act0:QCOLS], exp, scale=SCALE
                )
                return u, states

            def emit_consume(b, c, u, states):
                last = [last_chunk(b, n) for n in range(2)]
                for n in range(2):
                    if states[n] == "skip":
                        continue
                    a = n * NHALF + clip_a(b, c, n)
                    hi = (n + 1) * NHALF
                    nc.tensor.matmul(
                        l_bc[:, a:hi],
                        ones_t[:],
                        u[:, a:hi],
                        start=c == 0,
                        stop=c == last[n],
                        skip_group_check=True,
                    )
                for n in range(2):
                    if states[n] == "skip":
                        continue
                    a = n * NHALF + clip_a(b, c, n)
                    hi = (n + 1) * NHALF
                    nc.tensor.matmul(
                        o_ps[:, a:hi],
                        v_t[b][:, c * CHUNK : (c + 1) * CHUNK],
                        u[:, a:hi],
                        start=c == 0,
                        stop=c == last[n],
                        skip_group_check=True,
                    )

            def emit_epilogue_half(b, n, terminal):
                half = slice(n * NHALF, (n + 1) * NHALF)
                rl = sbe.tile([CHUNK, QCOLS], F32, tag=f"rl{n}")
                nc.vector.reciprocal_approx_fast(rl[:, half], l_bc[:, half])
                if terminal:
                    osrc = o_ps
                else:
                    osrc = sbe.tile([D, QCOLS], F32, tag=f"ocp{n}")
                    nc.vector.tensor_copy(osrc[:, half], o_ps[:, half])
                out_sb = sbe.tile([D, QCOLS], F16, tag=f"osb{n}")
                nc.vector.tensor_mul(
                    out_sb[:, half], osrc[:, half], rl[:, half]
                )
                nc.sync.dma_start(out_d.ap()[b][:, half], out_sb[:, half])

            flat = [(b, c) for b in border for c in range(cb[b])]
            pend = None
            for i, (b, c) in enumerate(flat):
                u, states = emit_score(b, c)
                if pend is not None:
                    pb, pc, pu, pst = pend
                    emit_consume(pb, pc, pu, pst)
                    if pc == cb[pb] - 1:
                        emit_epilogue_half(pb, 0, terminal=False)
                        emit_epilogue_half(pb, 1, terminal=False)
                pend = (b, c, u, states)
            pb, pc, pu, pst = pend
            # terminal: half 0 finishes a chunk early -- emit its epilogue
            # pieces as soon as its l/O accumulation stops.
            emit_consume(pb, pc, pu, pst)
            emit_epilogue_half(pb, 0, terminal=True)
            emit_epilogue_half(pb, 1, terminal=True)

    nc.compile()
    return nc, L, cb, offs


def _pack_inputs(query, k_cache, v_cache, block_tables, L, cb, offs):
    """Gather the paged cache and pack per-core fp16 shards in device layouts."""
    C = int(offs[-1])
    k_lin = k_cache[block_tables].reshape(B, KV, KVH, D)
    v_lin = v_cache[block_tables].reshape(B, KV, KVH, D)
    kt_all = np.zeros((KVH, D, C * CHUNK), dtype=np.float32)
    v_all = np.zeros((KVH, CHUNK, C * CHUNK), dtype=np.float32)
    for b in range(B):
        Lb, w = int(L[b]), cb[b] * CHUNK
        kk = np.zeros((w, KVH, D), dtype=np.float32)
        kk[:Lb] = k_lin[b, :Lb]
        kt_all[:, :, offs[b] * CHUNK : offs[b] * CHUNK + w] = kk.transpose(
            1, 2, 0
        )
        vv = np.zeros((w, KVH, D), dtype=np.float32)
        vv[:Lb] = v_lin[b, :Lb]
        v_all[:, :, offs[b] * CHUNK : offs[b] * CHUNK + w] = (
            vv.reshape(cb[b], CHUNK, KVH, D)
            .transpose(2, 1, 0, 3)
            .reshape(KVH, CHUNK, w)
        )
    # query [B,Q,H,D] -> [KVH, D, B, Q, G] (t-major, g inner)
    qt_all = (
        query.transpose(2, 3, 0, 1)
        .reshape(KVH, G, D, B, Q)
        .transpose(0, 2, 3, 4, 1)
        .reshape(KVH, D, B * QCOLS)
    )
    kt_all = kt_all.astype(np.float16)
    v_all = v_all.astype(np.float16)
    qt_all = np.ascontiguousarray(qt_all).astype(np.float16)
    return [
        {
            "kt": np.ascontiguousarray(kt_all[h]),
            "v": np.ascontiguousarray(v_all[h]),
            "qt": qt_all[h],
        }
        for h in range(KVH)
    ]


def _unpack_outputs(results):
    """[B,D,QCOLS] fp16 per core (O^T, q=(t,g) on cols) -> [B*Q, H*D] f32."""
    out = np.empty((B * Q, H * D), dtype=np.float32)
    for h, res in enumerate(results):
        o = res["out"].astype(np.float32).reshape(B, D, Q, G)  # [b, d, t, g]
        o = o.transpose(0, 2, 3, 1).reshape(B * Q, G * D)
        out[:, h * G * D : (h + 1) * G * D] = o
    return out


def kernel(query, k_cache, v_cache, block_tables, seq_lens):
    query = np.asarray(query, dtype=np.float32)
    k_cache = np.asarray(k_cache, dtype=np.float32)
    v_cache = np.asarray(v_cache, dtype=np.float32)
    block_tables = np.asarray(block_tables, dtype=np.int64)
    nc, L, cb, offs = _build(np.asarray(seq_lens))
    in_maps = _pack_inputs(query, k_cache, v_cache, block_tables, L, cb, offs)
    res = run_bass_kernel_spmd(nc, in_maps, core_ids=list(range(N_CORES)))
    return _unpack_outputs(res.results)


# revision 17
# speedup vs baseline: 1.1351x; 1.0036x over previous
"""Paged GQA chunked-prefill attention for 8 Trainium2 NeuronCores.

Problem (hardcoded): B=4 seqs x Q=256 new tokens, H=32 query heads, KVH=8 kv
heads (GQA group G=4), D=128 head dim, paged KV cache of 512 blocks x 16
tokens, per-seq lengths in seq_lens (clamped to >= Q), causal masking.

Sharding: tensor-parallel over heads. Core h gets kv head h and query heads
h*4..h*4+3; block_tables/seq_lens are resolved host-side while packing the
shards; the output is all-gathered host-side over the hidden dim.

Per-core device kernel, all matmul operands fp16 (full PE rate, fp32 PSUM
accumulation). For seq b, kv chunk c of 128 positions, q = (t,g) -> 1024
columns in two 512-column halves n; fully-masked column prefixes of boundary
chunks are clipped out of every stage:
  S^T[kv,q] = K_c^T q                 (fp16 matmul into PSUM)
  S^T += causal mask                  (identity-lhsT matmul, boundary band)
  U = exp(SCALE * S^T)                (one ScalarE pass over the active cols)
  lb[128,q] += ones128^T @ U          (all-ones lhsT: the denominator lands
                                       broadcast across all 128 partitions,
                                       so the epilogue needs no cross-
                                       partition moves at all)
  O^T[d,q] += V_c^T @ U               (PSUM accumulation over chunks)
Scores/exp are emitted one chunk ahead of l/PV consumption so the PE never
stalls behind the ScalarE exp. Epilogue per half: rl = 1/lb (fast approx
reciprocal straight out of PSUM), out = O^T * rl -> fp16, DMA out. The PE is
kept warm from t=0 with matmuls on a memset tile while input DMAs stream.
"""
import math

import numpy as np

import concourse.mybir as mybir
import concourse.tile as tile
from concourse import bacc
from concourse.bass_utils import run_bass_kernel_spmd

B, Q, H, D = 4, 256, 32, 128
KVH = 8
G = H // KVH
BLOCK = 16
NB = 128
KV = NB * BLOCK
NUM_BLOCKS = B * NB
SCALE = 1.0 / math.sqrt(D)
N_CORES = 8
CHUNK = 128
QCOLS = G * Q  # 1024 q columns per sequence per core
NHALF = 512

F32 = mybir.dt.float32
F16 = mybir.dt.float16
NEG = -60000.0  # exactly representable in fp16; SCALE*NEG ~ -5300 -> exp = 0


def _plan(seq_lens):
    """Chunk counts, processing order, and tight boundary mask tiles."""
    L = np.maximum(np.asarray(seq_lens, dtype=np.int64), Q)
    cb = [int((int(x) + CHUNK - 1) // CHUNK) for x in L]
    offs = np.concatenate([[0], np.cumsum(cb)]).astype(int)
    border = sorted(range(B), key=lambda b: cb[b])  # shortest first
    # (b, c, n) -> (tmin, tup, mask[128, tup-tmin]); cols t < tmin are fully
    # masked (clipped everywhere), t >= tup fully visible.
    masked = {}
    p = np.arange(CHUNK)
    for b in range(B):
        for c in range(cb[b]):
            for n in range(2):
                lo = int(L[b]) - Q + n * CHUNK  # kv pos of this half's t=0
                if c * CHUNK > lo + CHUNK - 1:
                    continue  # fully masked half
                if c * CHUNK + CHUNK - 1 > lo:
                    tmin = max(0, min(CHUNK, c * CHUNK - lo))
                    tup = max(0, min(CHUNK, c * CHUNK + CHUNK - lo))
                    t = np.arange(tmin, tup)
                    kvpos = c * CHUNK + p
                    m = np.where(
                        kvpos[:, None] > lo + t[None, :], NEG, 0.0
                    ).astype(np.float16)
                    masked[(b, c, n)] = (tmin, tup, m)
    order = sorted(
        masked.keys(), key=lambda k: (border.index(k[0]), k[1], k[2])
    )
    return L, cb, offs, border, masked, order


def _build(seq_lens):
    L, cb, offs, border, masked, order = _plan(seq_lens)
    C = int(offs[-1])
    mask_np = np.concatenate(
        [masked[k][2] for k in order] or [np.zeros((CHUNK, 1), np.float16)],
        axis=1,
    )
    mcols = mask_np.shape[1]
    moff = {}
    acc = 0
    for k in order:
        moff[k] = acc
        acc += masked[k][2].shape[1]
    ident_np = np.eye(CHUNK, dtype=np.float16)
    ones_np = np.ones((CHUNK, CHUNK), dtype=np.float16)

    nc = bacc.Bacc(
        "TRN2", target_bir_lowering=False, debug=False, num_devices=N_CORES
    )
    kt_d = nc.dram_tensor("kt", [D, C * CHUNK], F16, kind="ExternalInput")
    v_d = nc.dram_tensor("v", [CHUNK, C * CHUNK], F16, kind="ExternalInput")
    qt_d = nc.dram_tensor("qt", [D, B * QCOLS], F16, kind="ExternalInput")
    out_d = nc.dram_tensor("out", [B, D, QCOLS], F16, kind="ExternalOutput")
    mask_d = nc.inline_tensor(mask_np, name="mask_const")
    ident_d = nc.inline_tensor(ident_np, name="ident_const")
    ones_d = nc.inline_tensor(ones_np, name="ones_const")

    exp = mybir.ActivationFunctionType.Exp

    def half_lo(b, n):
        return int(L[b]) - Q + n * CHUNK

    def half_state(b, c, n):
        if c * CHUNK > half_lo(b, n) + CHUNK - 1:
            return "skip"
        if (b, c, n) in masked:
            return "mask"
        return "clear"

    def last_chunk(b, n):
        return min(cb[b] - 1, (half_lo(b, n) + CHUNK - 1) // CHUNK)

    def clip_a(b, c, n):
        """First active column (of 512) for this chunk-half."""
        if (b, c, n) in masked:
            return G * masked[(b, c, n)][0]
        return 0

    with tile.TileContext(nc) as tc:
        with (
            tc.tile_pool(name="sbin", bufs=1) as sbin,
            tc.tile_pool(name="sbu", bufs=4) as sbu,
            tc.tile_pool(name="sbe", bufs=2) as sbe,
            tc.tile_pool(name="ps_s", bufs=2, space="PSUM") as ps_s,
            tc.tile_pool(name="ps_o", bufs=1, space="PSUM") as ps_o,
            tc.tile_pool(name="ps_l", bufs=1, space="PSUM") as ps_l,
        ):
            # Warm the PE + load the exp table before any DMA lands: matmuls
            # on a memset tile keep the HAM activity window busy so the clock
            # is at 2.4 GHz when real data arrives.
            warm_w = sbin.tile([CHUNK, CHUNK], F16, tag="warmw")
            nc.vector.memset(warm_w[:], 0.0)
            dummy = sbe.tile([CHUNK, 1], F32, tag="dummy")
            nc.scalar.activation(dummy[:], warm_w[:, 0:1], exp)
            # l broadcast accumulators double as the warmup target; the first
            # real l matmul (start=True) resets them.
            l_bc = ps_l.tile([CHUNK, QCOLS], F32, tag="lbc")
            for _ in range(30):
                nc.tensor.matmul(
                    l_bc[:, 0:CHUNK], warm_w[:], warm_w[:],
                    start=True, stop=True, skip_group_check=True,
                )

            # Input DMAs in processing order; first sequence split fine.
            b0 = border[0]
            kt_t = [None] * B
            v_t = [None] * B
            qt_t = [None] * B
            w0 = cb[b0] * CHUNK
            qt0 = sbin.tile([D, QCOLS], F16, tag=f"qt{b0}")
            nc.sync.dma_start(
                qt0[:, 0:NHALF], qt_d.ap()[:, b0 * QCOLS : b0 * QCOLS + NHALF]
            )
            # first-seq loads split across the two HW-DGE rings (SP + ACT)
            # so their issue costs don't serialize.
            kt0 = sbin.tile([D, w0], F16, tag=f"kt{b0}")
            nc.scalar.dma_start(
                kt0[:], kt_d.ap()[:, offs[b0] * CHUNK : offs[b0] * CHUNK + w0]
            )
            v0 = sbin.tile([CHUNK, w0], F16, tag=f"v{b0}")
            nc.scalar.dma_start(
                v0[:], v_d.ap()[:, offs[b0] * CHUNK : offs[b0] * CHUNK + w0]
            )
            nc.sync.dma_start(
                qt0[:, NHALF:QCOLS],
                qt_d.ap()[:, b0 * QCOLS + NHALF : (b0 + 1) * QCOLS],
            )
            identr = sbin.tile([CHUNK, CHUNK], F16, tag="identr")
            nc.scalar.dma_start(identr[:], ident_d.ap())
            ones_t = sbin.tile([CHUNK, CHUNK], F16, tag="ones")
            nc.scalar.dma_start(ones_t[:], ones_d.ap())
            kt_t[b0], v_t[b0], qt_t[b0] = kt0, v0, qt0

            masks = sbin.tile([CHUNK, mcols], F16, tag="masks")
            cut = sum(
                masked[k][2].shape[1]
                for k in order
                if border.index(k[0]) <= 1
            )
            cut = max(1, min(cut, mcols))
            nc.sync.dma_start(masks[:, 0:cut], mask_d.ap()[:, 0:cut])

            for b in border[1:]:
                w = cb[b] * CHUNK
                o0 = offs[b] * CHUNK
                qt = sbin.tile([D, QCOLS], F16, tag=f"qt{b}")
                nc.sync.dma_start(qt[:], qt_d.ap()[:, b * QCOLS : (b + 1) * QCOLS])
                kt = sbin.tile([D, w], F16, tag=f"kt{b}")
                vt = sbin.tile([CHUNK, w], F16, tag=f"v{b}")
                head = min(4 * CHUNK, w)
                nc.sync.dma_start(kt[:, 0:head], kt_d.ap()[:, o0 : o0 + head])
                nc.sync.dma_start(vt[:, 0:head], v_d.ap()[:, o0 : o0 + head])
                if head < w:
                    nc.sync.dma_start(
                        kt[:, head:w], kt_d.ap()[:, o0 + head : o0 + w]
                    )
                    nc.sync.dma_start(
                        vt[:, head:w], v_d.ap()[:, o0 + head : o0 + w]
                    )
                kt_t[b], v_t[b], qt_t[b] = kt, vt, qt
            if cut < mcols:
                nc.sync.dma_start(
                    masks[:, cut:mcols], mask_d.ap()[:, cut:mcols]
                )

            o_ps = ps_o.tile([D, QCOLS], F32, tag="o")

            def emit_score(b, c):
                states = [half_state(b, c, n) for n in range(2)]
                s_ps = ps_s.tile([CHUNK, QCOLS], F32, tag="s")
                for n in range(2):
                    if states[n] == "skip":
                        continue
                    h0 = n * NHALF
                    if states[n] == "clear":
                        nc.tensor.matmul(
                            s_ps[:, h0 : h0 + NHALF],
                            kt_t[b][:, c * CHUNK : (c + 1) * CHUNK],
                            qt_t[b][:, h0 : h0 + NHALF],
                            start=True,
                            stop=True,
                        )
                        continue
                    tmin, tup, _ = masked[(b, c, n)]
                    a, u_c = G * tmin, G * tup
                    nc.tensor.matmul(
                        s_ps[:, h0 + a : h0 + u_c],
                        kt_t[b][:, c * CHUNK : (c + 1) * CHUNK],
                        qt_t[b][:, h0 + a : h0 + u_c],
                        start=True,
                        stop=False,
                    )
                    mo = moff[(b, c, n)]
                    mb = (
                        masks[:, mo : mo + (tup - tmin)]
                        .unsqueeze(2)
                        .broadcast_to([CHUNK, tup - tmin, G])
                    )
                    nc.tensor.matmul(
                        s_ps[:, h0 + a : h0 + u_c],
                        identr[:],
                        mb,
                        start=False,
                        stop=True,
                    )
                    if u_c < NHALF:
                        nc.tensor.matmul(
                            s_ps[:, h0 + u_c : h0 + NHALF],
                            kt_t[b][:, c * CHUNK : (c + 1) * CHUNK],
                            qt_t[b][:, h0 + u_c : h0 + NHALF],
                            start=True,
                            stop=True,
                        )
                act0 = (
                    clip_a(b, c, 0)
                    if states[0] != "skip"
                    else NHALF + clip_a(b, c, 1)
                )
                u = sbu.tile([CHUNK, QCOLS], F16, tag="u")
                nc.scalar.activation(
                    u[:, act0:QCOLS], s_ps[:, act0:QCOLS], exp, scale=SCALE
                )
                return u, states

            def emit_consume(b, c, u, states):
                last = [last_chunk(b, n) for n in range(2)]
                for n in range(2):
                    if states[n] == "skip":
                        continue
                    a = n * NHALF + clip_a(b, c, n)
                    hi = (n + 1) * NHALF
                    nc.tensor.matmul(
                        l_bc[:, a:hi],
                        ones_t[:],
                        u[:, a:hi],
                        start=c == 0,
                        stop=c == last[n],
                        skip_group_check=True,
                    )
                for n in range(2):
                    if states[n] == "skip":
                        continue
                    a = n * NHALF + clip_a(b, c, n)
                    hi = (n + 1) * NHALF
                    nc.tensor.matmul(
                        o_ps[:, a:hi],
                        v_t[b][:, c * CHUNK : (c + 1) * CHUNK],
                        u[:, a:hi],
                        start=c == 0,
                        stop=c == last[n],
                        skip_group_check=True,
                    )

            def emit_epilogue_half(b, n, terminal):
                half = slice(n * NHALF, (n + 1) * NHALF)
                rl = sbe.tile([CHUNK, QCOLS], F32, tag=f"rl{n}")
                nc.vector.reciprocal_approx_fast(rl[:, half], l_bc[:, half])
                if terminal:
                    osrc = o_ps
                else:
                    osrc = sbe.tile([D, QCOLS], F32, tag=f"ocp{n}")
                    nc.vector.tensor_copy(osrc[:, half], o_ps[:, half])
                out_sb = sbe.tile([D, QCOLS], F16, tag=f"osb{n}")
                nc.vector.tensor_mul(
                    out_sb[:, half], osrc[:, half], rl[:, half]
                )
                nc.sync.dma_start(out_d.ap()[b][:, half], out_sb[:, half])

            # The final chunk of every sequence always skips half 0 (its last
            # contributing chunk is earlier), so half 0's epilogue can be
            # emitted BEFORE the final chunk's l/PV consumption.
            def flush(pb, pc, pu, pst, terminal):
                if pc == cb[pb] - 1:
                    emit_epilogue_half(pb, 0, terminal=terminal)
                    emit_consume(pb, pc, pu, pst)
                    emit_epilogue_half(pb, 1, terminal=terminal)
                else:
                    emit_consume(pb, pc, pu, pst)

            flat = [(b, c) for b in border for c in range(cb[b])]
            pend = None
            for b, c in flat:
                u, states = emit_score(b, c)
                if pend is not None:
                    pb, pc, pu, pst = pend
                    flush(pb, pc, pu, pst, terminal=False)
                pend = (b, c, u, states)
            pb, pc, pu, pst = pend
            flush(pb, pc, pu, pst, terminal=True)

    nc.compile()
    return nc, L, cb, offs


def _pack_inputs(query, k_cache, v_cache, block_tables, L, cb, offs):
    """Gather the paged cache and pack per-core fp16 shards in device layouts."""
    C = int(offs[-1])
    k_lin = k_cache[block_tables].reshape(B, KV, KVH, D)
    v_lin = v_cache[block_tables].reshape(B, KV, KVH, D)
    kt_all = np.zeros((KVH, D, C * CHUNK), dtype=np.float32)
    v_all = np.zeros((KVH, CHUNK, C * CHUNK), dtype=np.float32)
    for b in range(B):
        Lb, w = int(L[b]), cb[b] * CHUNK
        kk = np.zeros((w, KVH, D), dtype=np.float32)
        kk[:Lb] = k_lin[b, :Lb]
        kt_all[:, :, offs[b] * CHUNK : offs[b] * CHUNK + w] = kk.transpose(
            1, 2, 0
        )
        vv = np.zeros((w, KVH, D), dtype=np.float32)
        vv[:Lb] = v_lin[b, :Lb]
        v_all[:, :, offs[b] * CHUNK : offs[b] * CHUNK + w] = (
            vv.reshape(cb[b], CHUNK, KVH, D)
            .transpose(2, 1, 0, 3)
            .reshape(KVH, CHUNK, w)
        )
    # query [B,Q,H,D] -> [KVH, D, B, Q, G] (t-major, g inner)
    qt_all = (
        query.transpose(2, 3, 0, 1)
        .reshape(KVH, G, D, B, Q)
        .transpose(0, 2, 3, 4, 1)
        .reshape(KVH, D, B * QCOLS)
    )
    kt_all = kt_all.astype(np.float16)
    v_all = v_all.astype(np.float16)
    qt_all = np.ascontiguousarray(qt_all).astype(np.float16)
    return [
        {
            "kt": np.ascontiguousarray(kt_all[h]),
            "v": np.ascontiguousarray(v_all[h]),
            "qt": qt_all[h],
        }
        for h in range(KVH)
    ]


def _unpack_outputs(results):
    """[B,D,QCOLS] fp16 per core (O^T, q=(t,g) on cols) -> [B*Q, H*D] f32."""
    out = np.empty((B * Q, H * D), dtype=np.float32)
    for h, res in enumerate(results):
        o = res["out"].astype(np.float32).reshape(B, D, Q, G)  # [b, d, t, g]
        o = o.transpose(0, 2, 3, 1).reshape(B * Q, G * D)
        out[:, h * G * D : (h + 1) * G * D] = o
    return out


def kernel(query, k_cache, v_cache, block_tables, seq_lens):
    query = np.asarray(query, dtype=np.float32)
    k_cache = np.asarray(k_cache, dtype=np.float32)
    v_cache = np.asarray(v_cache, dtype=np.float32)
    block_tables = np.asarray(block_tables, dtype=np.int64)
    nc, L, cb, offs = _build(np.asarray(seq_lens))
    in_maps = _pack_inputs(query, k_cache, v_cache, block_tables, L, cb, offs)
    res = run_bass_kernel_spmd(nc, in_maps, core_ids=list(range(N_CORES)))
    return _unpack_outputs(res.results)


# revision 18
# speedup vs baseline: 1.2064x; 1.0628x over previous
"""Paged GQA chunked-prefill attention for 8 Trainium2 NeuronCores.

Problem (hardcoded): B=4 seqs x Q=256 new tokens, H=32 query heads, KVH=8 kv
heads (GQA group G=4), D=128 head dim, paged KV cache of 512 blocks x 16
tokens, per-seq lengths in seq_lens (clamped to >= Q), causal masking.

Sharding: tensor-parallel over heads. Core h gets kv head h and query heads
h*4..h*4+3; block_tables/seq_lens are resolved host-side while packing the
shards; the output is all-gathered host-side over the hidden dim.

Per-core device kernel, all matmul operands fp16 (full PE rate, fp32 PSUM
accumulation). For seq b, kv chunk c of 128 positions, q = (t,g) -> 1024
columns in two 512-column halves n; fully-masked column prefixes of boundary
chunks are clipped out of every stage:
  S^T[kv,q] = K_c^T q                 (fp16 matmul into PSUM)
  S^T += causal mask                  (identity-lhsT matmul, boundary band)
  U = exp(SCALE * S^T)                (one ScalarE pass over the active cols)
  lb[128,q] += ones128^T @ U          (all-ones lhsT: the denominator lands
                                       broadcast across all 128 partitions,
                                       so the epilogue needs no cross-
                                       partition moves at all)
  O^T[d,q] += V_c^T @ U               (PSUM accumulation over chunks)
Scores/exp are emitted one chunk ahead of l/PV consumption so the PE never
stalls behind the ScalarE exp. Epilogue per half: rl = 1/lb (fast approx
reciprocal straight out of PSUM), out = O^T * rl -> fp16, DMA out. The PE is
kept warm from t=0 with matmuls on a memset tile while input DMAs stream.
"""
import math

import numpy as np

import concourse.mybir as mybir
import concourse.tile as tile
from concourse import bacc
from concourse.bass_utils import run_bass_kernel_spmd

B, Q, H, D = 4, 256, 32, 128
KVH = 8
G = H // KVH
BLOCK = 16
NB = 128
KV = NB * BLOCK
NUM_BLOCKS = B * NB
SCALE = 1.0 / math.sqrt(D)
N_CORES = 8
CHUNK = 128
QCOLS = G * Q  # 1024 q columns per sequence per core
NHALF = 512

F32 = mybir.dt.float32
F16 = mybir.dt.float16
NEG = -60000.0  # exactly representable in fp16; SCALE*NEG ~ -5300 -> exp = 0


def _plan(seq_lens):
    """Chunk counts, processing order, and tight boundary mask tiles."""
    L = np.maximum(np.asarray(seq_lens, dtype=np.int64), Q)
    cb = [int((int(x) + CHUNK - 1) // CHUNK) for x in L]
    offs = np.concatenate([[0], np.cumsum(cb)]).astype(int)
    border = sorted(range(B), key=lambda b: -cb[b])  # longest first
    # (b, c, n) -> (tmin, tup, mask[128, tup-tmin]); cols t < tmin are fully
    # masked (clipped everywhere), t >= tup fully visible.
    masked = {}
    p = np.arange(CHUNK)
    for b in range(B):
        for c in range(cb[b]):
            for n in range(2):
                lo = int(L[b]) - Q + n * CHUNK  # kv pos of this half's t=0
                if c * CHUNK > lo + CHUNK - 1:
                    continue  # fully masked half
                if c * CHUNK + CHUNK - 1 > lo:
                    tmin = max(0, min(CHUNK, c * CHUNK - lo))
                    tup = max(0, min(CHUNK, c * CHUNK + CHUNK - lo))
                    t = np.arange(tmin, tup)
                    kvpos = c * CHUNK + p
                    m = np.where(
                        kvpos[:, None] > lo + t[None, :], NEG, 0.0
                    ).astype(np.float16)
                    masked[(b, c, n)] = (tmin, tup, m)
    order = sorted(
        masked.keys(), key=lambda k: (border.index(k[0]), k[1], k[2])
    )
    return L, cb, offs, border, masked, order


def _build(seq_lens):
    L, cb, offs, border, masked, order = _plan(seq_lens)
    C = int(offs[-1])
    mask_np = np.concatenate(
        [masked[k][2] for k in order] or [np.zeros((CHUNK, 1), np.float16)],
        axis=1,
    )
    mcols = mask_np.shape[1]
    moff = {}
    acc = 0
    for k in order:
        moff[k] = acc
        acc += masked[k][2].shape[1]
    ident_np = np.eye(CHUNK, dtype=np.float16)
    ones_np = np.ones((CHUNK, CHUNK), dtype=np.float16)

    nc = bacc.Bacc(
        "TRN2", target_bir_lowering=False, debug=False, num_devices=N_CORES
    )
    kt_d = nc.dram_tensor("kt", [D, C * CHUNK], F16, kind="ExternalInput")
    v_d = nc.dram_tensor("v", [CHUNK, C * CHUNK], F16, kind="ExternalInput")
    qt_d = nc.dram_tensor("qt", [D, B * QCOLS], F16, kind="ExternalInput")
    out_d = nc.dram_tensor("out", [B, D, QCOLS], F16, kind="ExternalOutput")
    mask_d = nc.inline_tensor(mask_np, name="mask_const")
    ident_d = nc.inline_tensor(ident_np, name="ident_const")
    ones_d = nc.inline_tensor(ones_np, name="ones_const")

    exp = mybir.ActivationFunctionType.Exp

    def half_lo(b, n):
        return int(L[b]) - Q + n * CHUNK

    def half_state(b, c, n):
        if c * CHUNK > half_lo(b, n) + CHUNK - 1:
            return "skip"
        if (b, c, n) in masked:
            return "mask"
        return "clear"

    def last_chunk(b, n):
        return min(cb[b] - 1, (half_lo(b, n) + CHUNK - 1) // CHUNK)

    def clip_a(b, c, n):
        """First active column (of 512) for this chunk-half."""
        if (b, c, n) in masked:
            return G * masked[(b, c, n)][0]
        return 0

    with tile.TileContext(nc) as tc:
        with (
            tc.tile_pool(name="sbin", bufs=1) as sbin,
            tc.tile_pool(name="sbu", bufs=4) as sbu,
            tc.tile_pool(name="sbe", bufs=2) as sbe,
            tc.tile_pool(name="ps_s", bufs=2, space="PSUM") as ps_s,
            tc.tile_pool(name="ps_o", bufs=1, space="PSUM") as ps_o,
            tc.tile_pool(name="ps_l", bufs=1, space="PSUM") as ps_l,
        ):
            # Warm the PE + load the exp table before any DMA lands: matmuls
            # on a memset tile keep the HAM activity window busy so the clock
            # is at 2.4 GHz when real data arrives.
            warm_w = sbin.tile([CHUNK, CHUNK], F16, tag="warmw")
            nc.vector.memset(warm_w[:], 0.0)
            dummy = sbe.tile([CHUNK, 1], F32, tag="dummy")
            nc.scalar.activation(dummy[:], warm_w[:, 0:1], exp)
            # l broadcast accumulators (one tile per half so the early
            # half-0 epilogue never false-shares with half-1 matmuls); the
            # first tile doubles as the warmup target -- the first real
            # l matmul (start=True) resets it.
            l_bc0 = ps_l.tile([CHUNK, NHALF], F32, tag="lbc0")
            l_bc1 = ps_l.tile([CHUNK, NHALF], F32, tag="lbc1")
            l_bc = (l_bc0, l_bc1)
            for _ in range(30):
                nc.tensor.matmul(
                    l_bc0[:, 0:CHUNK], warm_w[:], warm_w[:],
                    start=True, stop=True, skip_group_check=True,
                )

            # Input DMAs in processing order (longest seq first). First-seq
            # loads split across the two HW-DGE rings (SP + ACT) so their
            # issue costs don't serialize; head chunks land first so compute
            # can start while the rest of the long sequence streams.
            b0 = border[0]
            kt_t = [None] * B
            v_t = [None] * B
            qt_t = [None] * B
            w0 = cb[b0] * CHUNK
            o0_ = offs[b0] * CHUNK
            head0 = min(2 * CHUNK, w0)
            kt0 = sbin.tile([D, w0], F16, tag=f"kt{b0}")
            nc.scalar.dma_start(kt0[:, 0:head0], kt_d.ap()[:, o0_ : o0_ + head0])
            qt0 = sbin.tile([D, QCOLS], F16, tag=f"qt{b0}")
            nc.sync.dma_start(
                qt0[:, 0:NHALF], qt_d.ap()[:, b0 * QCOLS : b0 * QCOLS + NHALF]
            )
            v0 = sbin.tile([CHUNK, w0], F16, tag=f"v{b0}")
            nc.scalar.dma_start(v0[:, 0:head0], v_d.ap()[:, o0_ : o0_ + head0])
            nc.sync.dma_start(
                qt0[:, NHALF:QCOLS],
                qt_d.ap()[:, b0 * QCOLS + NHALF : (b0 + 1) * QCOLS],
            )
            ones_t = sbin.tile([CHUNK, CHUNK], F16, tag="ones")
            nc.scalar.dma_start(ones_t[:], ones_d.ap())
            identr = sbin.tile([CHUNK, CHUNK], F16, tag="identr")
            nc.scalar.dma_start(identr[:], ident_d.ap())
            if head0 < w0:
                mid0 = (head0 + w0 + CHUNK) // (2 * CHUNK) * CHUNK
                nc.sync.dma_start(
                    kt0[:, head0:mid0], kt_d.ap()[:, o0_ + head0 : o0_ + mid0]
                )
                nc.scalar.dma_start(
                    v0[:, head0:mid0], v_d.ap()[:, o0_ + head0 : o0_ + mid0]
                )
                if mid0 < w0:
                    nc.sync.dma_start(
                        kt0[:, mid0:w0], kt_d.ap()[:, o0_ + mid0 : o0_ + w0]
                    )
                    nc.scalar.dma_start(
                        v0[:, mid0:w0], v_d.ap()[:, o0_ + mid0 : o0_ + w0]
                    )
            kt_t[b0], v_t[b0], qt_t[b0] = kt0, v0, qt0

            masks = sbin.tile([CHUNK, mcols], F16, tag="masks")
            cut = sum(
                masked[k][2].shape[1]
                for k in order
                if border.index(k[0]) <= 1
            )
            cut = max(1, min(cut, mcols))
            nc.sync.dma_start(masks[:, 0:cut], mask_d.ap()[:, 0:cut])

            for b in border[1:]:
                w = cb[b] * CHUNK
                o0 = offs[b] * CHUNK
                qt = sbin.tile([D, QCOLS], F16, tag=f"qt{b}")
                nc.sync.dma_start(qt[:], qt_d.ap()[:, b * QCOLS : (b + 1) * QCOLS])
                kt = sbin.tile([D, w], F16, tag=f"kt{b}")
                vt = sbin.tile([CHUNK, w], F16, tag=f"v{b}")
                head = min(4 * CHUNK, w)
                nc.sync.dma_start(kt[:, 0:head], kt_d.ap()[:, o0 : o0 + head])
                nc.sync.dma_start(vt[:, 0:head], v_d.ap()[:, o0 : o0 + head])
                if head < w:
                    nc.sync.dma_start(
                        kt[:, head:w], kt_d.ap()[:, o0 + head : o0 + w]
                    )
                    nc.sync.dma_start(
                        vt[:, head:w], v_d.ap()[:, o0 + head : o0 + w]
                    )
                kt_t[b], v_t[b], qt_t[b] = kt, vt, qt
            if cut < mcols:
                nc.sync.dma_start(
                    masks[:, cut:mcols], mask_d.ap()[:, cut:mcols]
                )

            o_ps0 = ps_o.tile([D, NHALF], F32, tag="o0")
            o_ps1 = ps_o.tile([D, NHALF], F32, tag="o1")
            o_ps = (o_ps0, o_ps1)

            def emit_score(b, c):
                states = [half_state(b, c, n) for n in range(2)]
                s_ps = ps_s.tile([CHUNK, QCOLS], F32, tag="s")
                for n in range(2):
                    if states[n] == "skip":
                        continue
                    h0 = n * NHALF
                    if states[n] == "clear":
                        nc.tensor.matmul(
                            s_ps[:, h0 : h0 + NHALF],
                            kt_t[b][:, c * CHUNK : (c + 1) * CHUNK],
                            qt_t[b][:, h0 : h0 + NHALF],
                            start=True,
                            stop=True,
                        )
                        continue
                    tmin, tup, _ = masked[(b, c, n)]
                    a, u_c = G * tmin, G * tup
                    nc.tensor.matmul(
                        s_ps[:, h0 + a : h0 + u_c],
                        kt_t[b][:, c * CHUNK : (c + 1) * CHUNK],
                        qt_t[b][:, h0 + a : h0 + u_c],
                        start=True,
                        stop=False,
                    )
                    mo = moff[(b, c, n)]
                    mb = (
                        masks[:, mo : mo + (tup - tmin)]
                        .unsqueeze(2)
                        .broadcast_to([CHUNK, tup - tmin, G])
                    )
                    nc.tensor.matmul(
                        s_ps[:, h0 + a : h0 + u_c],
                        identr[:],
                        mb,
                        start=False,
                        stop=True,
                    )
                    if u_c < NHALF:
                        nc.tensor.matmul(
                            s_ps[:, h0 + u_c : h0 + NHALF],
                            kt_t[b][:, c * CHUNK : (c + 1) * CHUNK],
                            qt_t[b][:, h0 + u_c : h0 + NHALF],
                            start=True,
                            stop=True,
                        )
                act0 = (
                    clip_a(b, c, 0)
                    if states[0] != "skip"
                    else NHALF + clip_a(b, c, 1)
                )
                u = sbu.tile([CHUNK, QCOLS], F16, tag="u")
                nc.scalar.activation(
                    u[:, act0:QCOLS], s_ps[:, act0:QCOLS], exp, scale=SCALE
                )
                return u, states

            def emit_consume(b, c, u, states):
                last = [last_chunk(b, n) for n in range(2)]
                for n in range(2):
                    if states[n] == "skip":
                        continue
                    a = clip_a(b, c, n)
                    nc.tensor.matmul(
                        l_bc[n][:, a:NHALF],
                        ones_t[:],
                        u[:, n * NHALF + a : (n + 1) * NHALF],
                        start=c == 0,
                        stop=c == last[n],
                        skip_group_check=True,
                    )
                for n in range(2):
                    if states[n] == "skip":
                        continue
                    a = clip_a(b, c, n)
                    nc.tensor.matmul(
                        o_ps[n][:, a:NHALF],
                        v_t[b][:, c * CHUNK : (c + 1) * CHUNK],
                        u[:, n * NHALF + a : (n + 1) * NHALF],
                        start=c == 0,
                        stop=c == last[n],
                        skip_group_check=True,
                    )

            def emit_epilogue_half(b, n, terminal):
                half = slice(n * NHALF, (n + 1) * NHALF)
                rl = sbe.tile([CHUNK, NHALF], F32, tag=f"rl{n}")
                nc.vector.reciprocal_approx_fast(rl[:], l_bc[n][:])
                if terminal:
                    osrc = o_ps[n]
                else:
                    osrc = sbe.tile([D, NHALF], F32, tag=f"ocp{n}")
                    nc.vector.tensor_copy(osrc[:], o_ps[n][:])
                out_sb = sbe.tile([D, NHALF], F16, tag=f"osb{n}")
                nc.vector.tensor_mul(out_sb[:], osrc[:], rl[:])
                nc.sync.dma_start(out_d.ap()[b][:, half], out_sb[:])

            # The final chunk of every sequence always skips half 0 (its last
            # contributing chunk is earlier), so half 0's epilogue can be
            # emitted BEFORE the final chunk's l/PV consumption.
            def flush(pb, pc, pu, pst, terminal):
                if pc == cb[pb] - 1:
                    emit_epilogue_half(pb, 0, terminal=terminal)
                    emit_consume(pb, pc, pu, pst)
                    emit_epilogue_half(pb, 1, terminal=terminal)
                else:
                    emit_consume(pb, pc, pu, pst)

            flat = [(b, c) for b in border for c in range(cb[b])]
            pend = None
            for b, c in flat:
                u, states = emit_score(b, c)
                if pend is not None:
                    pb, pc, pu, pst = pend
                    flush(pb, pc, pu, pst, terminal=False)
                pend = (b, c, u, states)
            pb, pc, pu, pst = pend
            flush(pb, pc, pu, pst, terminal=True)

    nc.compile()
    return nc, L, cb, offs


def _pack_inputs(query, k_cache, v_cache, block_tables, L, cb, offs):
    """Gather the paged cache and pack per-core fp16 shards in device layouts."""
    C = int(offs[-1])
    k_lin = k_cache[block_tables].reshape(B, KV, KVH, D)
    v_lin = v_cache[block_tables].reshape(B, KV, KVH, D)
    kt_all = np.zeros((KVH, D, C * CHUNK), dtype=np.float32)
    v_all = np.zeros((KVH, CHUNK, C * CHUNK), dtype=np.float32)
    for b in range(B):
        Lb, w = int(L[b]), cb[b] * CHUNK
        kk = np.zeros((w, KVH, D), dtype=np.float32)
        kk[:Lb] = k_lin[b, :Lb]
        kt_all[:, :, offs[b] * CHUNK : offs[b] * CHUNK + w] = kk.transpose(
            1, 2, 0
        )
        vv = np.zeros((w, KVH, D), dtype=np.float32)
        vv[:Lb] = v_lin[b, :Lb]
        v_all[:, :, offs[b] * CHUNK : offs[b] * CHUNK + w] = (
            vv.reshape(cb[b], CHUNK, KVH, D)
            .transpose(2, 1, 0, 3)
            .reshape(KVH, CHUNK, w)
        )
    # query [B,Q,H,D] -> [KVH, D, B, Q, G] (t-major, g inner)
    qt_all = (
        query.transpose(2, 3, 0, 1)
        .reshape(KVH, G, D, B, Q)
        .transpose(0, 2, 3, 4, 1)
        .reshape(KVH, D, B * QCOLS)
    )
    kt_all = kt_all.astype(np.float16)
    v_all = v_all.astype(np.float16)
    qt_all = np.ascontiguousarray(qt_all).astype(np.float16)
    return [
        {
            "kt": np.ascontiguousarray(kt_all[h]),
            "v": np.ascontiguousarray(v_all[h]),
            "qt": qt_all[h],
        }
        for h in range(KVH)
    ]


def _unpack_outputs(results):
    """[B,D,QCOLS] fp16 per core (O^T, q=(t,g) on cols) -> [B*Q, H*D] f32."""
    out = np.empty((B * Q, H * D), dtype=np.float32)
    for h, res in enumerate(results):
        o = res["out"].astype(np.float32).reshape(B, D, Q, G)  # [b, d, t, g]
        o = o.transpose(0, 2, 3, 1).reshape(B * Q, G * D)
        out[:, h * G * D : (h + 1) * G * D] = o
    return out


def kernel(query, k_cache, v_cache, block_tables, seq_lens):
    query = np.asarray(query, dtype=np.float32)
    k_cache = np.asarray(k_cache, dtype=np.float32)
    v_cache = np.asarray(v_cache, dtype=np.float32)
    block_tables = np.asarray(block_tables, dtype=np.int64)
    nc, L, cb, offs = _build(np.asarray(seq_lens))
    in_maps = _pack_inputs(query, k_cache, v_cache, block_tables, L, cb, offs)
    res = run_bass_kernel_spmd(nc, in_maps, core_ids=list(range(N_CORES)))
    return _unpack_outputs(res.results)


# revision 19
# speedup vs baseline: 1.2392x; 1.0272x over previous
"""Paged GQA chunked-prefill attention for 8 Trainium2 NeuronCores.

Problem (hardcoded): B=4 seqs x Q=256 new tokens, H=32 query heads, KVH=8 kv
heads (GQA group G=4), D=128 head dim, paged KV cache of 512 blocks x 16
tokens, per-seq lengths in seq_lens (clamped to >= Q), causal masking.

Sharding: tensor-parallel over heads. Core h gets kv head h and query heads
h*4..h*4+3; block_tables/seq_lens are resolved host-side while packing the
shards; the output is all-gathered host-side over the hidden dim.

Per-core device kernel, all matmul operands fp16 (full PE rate, fp32 PSUM
accumulation). For seq b, kv chunk c of 128 positions, q = (t,g) -> 1024
columns in two 512-column halves n; fully-masked column prefixes of boundary
chunks are clipped out of every stage:
  S^T[kv,q] = K_c^T q                 (fp16 matmul into PSUM)
  S^T += causal mask                  (identity-lhsT matmul, boundary band)
  U = exp(SCALE * S^T)                (one ScalarE pass over the active cols)
  lb[128,q] += ones128^T @ U          (all-ones lhsT: the denominator lands
                                       broadcast across all 128 partitions,
                                       so the epilogue needs no cross-
                                       partition moves at all)
  O^T[d,q] += V_c^T @ U               (PSUM accumulation over chunks)
Scores/exp are emitted one chunk ahead of l/PV consumption so the PE never
stalls behind the ScalarE exp. Epilogue per half: rl = 1/lb (fast approx
reciprocal straight out of PSUM), out = O^T * rl -> fp16, DMA out. The PE is
kept warm from t=0 with matmuls on a memset tile while input DMAs stream.
"""
import math

import numpy as np

import concourse.mybir as mybir
import concourse.tile as tile
from concourse import bacc
from concourse.bass_utils import run_bass_kernel_spmd

B, Q, H, D = 4, 256, 32, 128
KVH = 8
G = H // KVH
BLOCK = 16
NB = 128
KV = NB * BLOCK
NUM_BLOCKS = B * NB
SCALE = 1.0 / math.sqrt(D)
N_CORES = 8
CHUNK = 128
QCOLS = G * Q  # 1024 q columns per sequence per core
NHALF = 512

F32 = mybir.dt.float32
F16 = mybir.dt.float16
NEG = -60000.0  # exactly representable in fp16; SCALE*NEG ~ -5300 -> exp = 0


def _plan(seq_lens):
    """Chunk counts, processing order, and tight boundary mask tiles."""
    L = np.maximum(np.asarray(seq_lens, dtype=np.int64), Q)
    cb = [int((int(x) + CHUNK - 1) // CHUNK) for x in L]
    offs = np.concatenate([[0], np.cumsum(cb)]).astype(int)
    border = sorted(range(B), key=lambda b: -cb[b])  # longest first
    # (b, c, n) -> (tmin, tup, mask[128, tup-tmin]); cols t < tmin are fully
    # masked (clipped everywhere), t >= tup fully visible.
    masked = {}
    p = np.arange(CHUNK)
    for b in range(B):
        for c in range(cb[b]):
            for n in range(2):
                lo = int(L[b]) - Q + n * CHUNK  # kv pos of this half's t=0
                if c * CHUNK > lo + CHUNK - 1:
                    continue  # fully masked half
                if c * CHUNK + CHUNK - 1 > lo:
                    tmin = max(0, min(CHUNK, c * CHUNK - lo))
                    tup = max(0, min(CHUNK, c * CHUNK + CHUNK - lo))
                    t = np.arange(tmin, tup)
                    kvpos = c * CHUNK + p
                    m = np.where(
                        kvpos[:, None] > lo + t[None, :], NEG, 0.0
                    ).astype(np.float16)
                    masked[(b, c, n)] = (tmin, tup, m)
    order = sorted(
        masked.keys(), key=lambda k: (border.index(k[0]), k[1], k[2])
    )
    return L, cb, offs, border, masked, order


def _build(seq_lens):
    L, cb, offs, border, masked, order = _plan(seq_lens)
    C = int(offs[-1])
    mask_np = np.concatenate(
        [masked[k][2] for k in order] or [np.zeros((CHUNK, 1), np.float16)],
        axis=1,
    )
    mcols = mask_np.shape[1]
    moff = {}
    acc = 0
    for k in order:
        moff[k] = acc
        acc += masked[k][2].shape[1]
    ident_np = np.eye(CHUNK, dtype=np.float16)
    ones_np = np.ones((CHUNK, CHUNK), dtype=np.float16)

    nc = bacc.Bacc(
        "TRN2", target_bir_lowering=False, debug=False, num_devices=N_CORES
    )
    kt_d = nc.dram_tensor("kt", [D, C * CHUNK], F16, kind="ExternalInput")
    v_d = nc.dram_tensor("v", [CHUNK, C * CHUNK], F16, kind="ExternalInput")
    qt_d = nc.dram_tensor("qt", [D, B * QCOLS], F16, kind="ExternalInput")
    out_d = nc.dram_tensor("out", [B, D, QCOLS], F16, kind="ExternalOutput")
    mask_d = nc.inline_tensor(mask_np, name="mask_const")
    ident_d = nc.inline_tensor(ident_np, name="ident_const")
    ones_d = nc.inline_tensor(ones_np, name="ones_const")

    exp = mybir.ActivationFunctionType.Exp

    def half_lo(b, n):
        return int(L[b]) - Q + n * CHUNK

    def half_state(b, c, n):
        if c * CHUNK > half_lo(b, n) + CHUNK - 1:
            return "skip"
        if (b, c, n) in masked:
            return "mask"
        return "clear"

    def last_chunk(b, n):
        return min(cb[b] - 1, (half_lo(b, n) + CHUNK - 1) // CHUNK)

    def clip_a(b, c, n):
        """First active column (of 512) for this chunk-half."""
        if (b, c, n) in masked:
            return G * masked[(b, c, n)][0]
        return 0

    with tile.TileContext(nc) as tc:
        with (
            tc.tile_pool(name="sbin", bufs=1) as sbin,
            tc.tile_pool(name="sbu", bufs=4) as sbu,
            tc.tile_pool(name="sbe", bufs=2) as sbe,
            tc.tile_pool(name="ps_s", bufs=2, space="PSUM") as ps_s,
            tc.tile_pool(name="ps_o", bufs=1, space="PSUM") as ps_o,
            tc.tile_pool(name="ps_l", bufs=1, space="PSUM") as ps_l,
        ):
            # Warm the PE + load the exp table before any DMA lands: matmuls
            # on a memset tile keep the HAM activity window busy so the clock
            # is at 2.4 GHz when real data arrives.
            warm_w = sbin.tile([CHUNK, CHUNK], F16, tag="warmw")
            nc.vector.memset(warm_w[:], 0.0)
            dummy = sbe.tile([CHUNK, 1], F32, tag="dummy")
            nc.scalar.activation(dummy[:], warm_w[:, 0:1], exp)
            # l broadcast accumulators (one tile per half so the early
            # half-0 epilogue never false-shares with half-1 matmuls); the
            # first tile doubles as the warmup target -- the first real
            # l matmul (start=True) resets it.
            l_bc0 = ps_l.tile([CHUNK, NHALF], F32, tag="lbc0")
            l_bc1 = ps_l.tile([CHUNK, NHALF], F32, tag="lbc1")
            l_bc = (l_bc0, l_bc1)
            for _ in range(30):
                nc.tensor.matmul(
                    l_bc0[:, 0:CHUNK], warm_w[:], warm_w[:],
                    start=True, stop=True, skip_group_check=True,
                )

            # Input DMAs in processing order (longest seq first). First-seq
            # loads split across the two HW-DGE rings (SP + ACT) so their
            # issue costs don't serialize; head chunks land first so compute
            # can start while the rest of the long sequence streams.
            b0 = border[0]
            kt_t = [None] * B
            v_t = [None] * B
            qt_t = [None] * B
            w0 = cb[b0] * CHUNK
            o0_ = offs[b0] * CHUNK
            head0 = min(2 * CHUNK, w0)
            kt0 = sbin.tile([D, w0], F16, tag=f"kt{b0}")
            nc.scalar.dma_start(kt0[:, 0:head0], kt_d.ap()[:, o0_ : o0_ + head0])
            qt0 = sbin.tile([D, QCOLS], F16, tag=f"qt{b0}")
            nc.sync.dma_start(
                qt0[:, 0:NHALF], qt_d.ap()[:, b0 * QCOLS : b0 * QCOLS + NHALF]
            )
            v0 = sbin.tile([CHUNK, w0], F16, tag=f"v{b0}")
            nc.scalar.dma_start(v0[:, 0:head0], v_d.ap()[:, o0_ : o0_ + head0])
            nc.sync.dma_start(
                qt0[:, NHALF:QCOLS],
                qt_d.ap()[:, b0 * QCOLS + NHALF : (b0 + 1) * QCOLS],
            )
            ones_t = sbin.tile([CHUNK, CHUNK], F16, tag="ones")
            nc.scalar.dma_start(ones_t[:], ones_d.ap())
            identr = sbin.tile([CHUNK, CHUNK], F16, tag="identr")
            nc.scalar.dma_start(identr[:], ident_d.ap())
            if head0 < w0:
                mid0 = (head0 + w0 + CHUNK) // (2 * CHUNK) * CHUNK
                nc.sync.dma_start(
                    kt0[:, head0:mid0], kt_d.ap()[:, o0_ + head0 : o0_ + mid0]
                )
                nc.scalar.dma_start(
                    v0[:, head0:mid0], v_d.ap()[:, o0_ + head0 : o0_ + mid0]
                )
                if mid0 < w0:
                    nc.sync.dma_start(
                        kt0[:, mid0:w0], kt_d.ap()[:, o0_ + mid0 : o0_ + w0]
                    )
                    nc.scalar.dma_start(
                        v0[:, mid0:w0], v_d.ap()[:, o0_ + mid0 : o0_ + w0]
                    )
            kt_t[b0], v_t[b0], qt_t[b0] = kt0, v0, qt0

            masks = sbin.tile([CHUNK, mcols], F16, tag="masks")
            cut = sum(
                masked[k][2].shape[1]
                for k in order
                if border.index(k[0]) <= 1
            )
            cut = max(1, min(cut, mcols))
            nc.sync.dma_start(masks[:, 0:cut], mask_d.ap()[:, 0:cut])

            for b in border[1:]:
                w = cb[b] * CHUNK
                o0 = offs[b] * CHUNK
                qt = sbin.tile([D, QCOLS], F16, tag=f"qt{b}")
                nc.sync.dma_start(qt[:], qt_d.ap()[:, b * QCOLS : (b + 1) * QCOLS])
                kt = sbin.tile([D, w], F16, tag=f"kt{b}")
                vt = sbin.tile([CHUNK, w], F16, tag=f"v{b}")
                head = min(4 * CHUNK, w)
                nc.sync.dma_start(kt[:, 0:head], kt_d.ap()[:, o0 : o0 + head])
                nc.sync.dma_start(vt[:, 0:head], v_d.ap()[:, o0 : o0 + head])
                if head < w:
                    nc.sync.dma_start(
                        kt[:, head:w], kt_d.ap()[:, o0 + head : o0 + w]
                    )
                    nc.sync.dma_start(
                        vt[:, head:w], v_d.ap()[:, o0 + head : o0 + w]
                    )
                kt_t[b], v_t[b], qt_t[b] = kt, vt, qt
            if cut < mcols:
                nc.sync.dma_start(
                    masks[:, cut:mcols], mask_d.ap()[:, cut:mcols]
                )

            o_ps0 = ps_o.tile([D, NHALF], F32, tag="o0")
            o_ps1 = ps_o.tile([D, NHALF], F32, tag="o1")
            o_ps = (o_ps0, o_ps1)

            def emit_score(b, c):
                states = [half_state(b, c, n) for n in range(2)]
                s_ps = ps_s.tile([CHUNK, QCOLS], F32, tag="s")
                for n in range(2):
                    if states[n] == "skip":
                        continue
                    h0 = n * NHALF
                    if states[n] == "clear":
                        nc.tensor.matmul(
                            s_ps[:, h0 : h0 + NHALF],
                            kt_t[b][:, c * CHUNK : (c + 1) * CHUNK],
                            qt_t[b][:, h0 : h0 + NHALF],
                            start=True,
                            stop=True,
                        )
                        continue
                    tmin, tup, _ = masked[(b, c, n)]
                    a, u_c = G * tmin, G * tup
                    nc.tensor.matmul(
                        s_ps[:, h0 + a : h0 + u_c],
                        kt_t[b][:, c * CHUNK : (c + 1) * CHUNK],
                        qt_t[b][:, h0 + a : h0 + u_c],
                        start=True,
                        stop=False,
                    )
                    mo = moff[(b, c, n)]
                    mb = (
                        masks[:, mo : mo + (tup - tmin)]
                        .unsqueeze(2)
                        .broadcast_to([CHUNK, tup - tmin, G])
                    )
                    nc.tensor.matmul(
                        s_ps[:, h0 + a : h0 + u_c],
                        identr[:],
                        mb,
                        start=False,
                        stop=True,
                    )
                    if u_c < NHALF:
                        nc.tensor.matmul(
                            s_ps[:, h0 + u_c : h0 + NHALF],
                            kt_t[b][:, c * CHUNK : (c + 1) * CHUNK],
                            qt_t[b][:, h0 + u_c : h0 + NHALF],
                            start=True,
                            stop=True,
                        )
                act0 = (
                    clip_a(b, c, 0)
                    if states[0] != "skip"
                    else NHALF + clip_a(b, c, 1)
                )
                u = sbu.tile([CHUNK, QCOLS], F16, tag="u")
                nc.scalar.activation(
                    u[:, act0:QCOLS], s_ps[:, act0:QCOLS], exp, scale=SCALE
                )
                return u, states

            def emit_consume(b, c, u, states):
                last = [last_chunk(b, n) for n in range(2)]
                for n in range(2):
                    if states[n] == "skip":
                        continue
                    a = clip_a(b, c, n)
                    nc.tensor.matmul(
                        l_bc[n][:, a:NHALF],
                        ones_t[:],
                        u[:, n * NHALF + a : (n + 1) * NHALF],
                        start=c == 0,
                        stop=c == last[n],
                        skip_group_check=True,
                    )
                    nc.tensor.matmul(
                        o_ps[n][:, a:NHALF],
                        v_t[b][:, c * CHUNK : (c + 1) * CHUNK],
                        u[:, n * NHALF + a : (n + 1) * NHALF],
                        start=c == 0,
                        stop=c == last[n],
                        skip_group_check=True,
                    )

            def emit_epilogue_half(b, n, terminal):
                half = slice(n * NHALF, (n + 1) * NHALF)
                rl = sbe.tile([CHUNK, NHALF], F32, tag=f"rl{n}")
                nc.vector.reciprocal_approx_fast(rl[:], l_bc[n][:])
                if terminal:
                    osrc = o_ps[n]
                else:
                    osrc = sbe.tile([D, NHALF], F32, tag=f"ocp{n}")
                    nc.vector.tensor_copy(osrc[:], o_ps[n][:])
                out_sb = sbe.tile([D, NHALF], F16, tag=f"osb{n}")
                nc.vector.tensor_mul(out_sb[:], osrc[:], rl[:])
                nc.sync.dma_start(out_d.ap()[b][:, half], out_sb[:])

            # The final chunk of every sequence always skips half 0 (its last
            # contributing chunk is earlier), so half 0's epilogue can be
            # emitted BEFORE the final chunk's l/PV consumption.
            def flush(pb, pc, pu, pst, terminal):
                if pc == cb[pb] - 1:
                    emit_epilogue_half(pb, 0, terminal=terminal)
                    emit_consume(pb, pc, pu, pst)
                    emit_epilogue_half(pb, 1, terminal=terminal)
                else:
                    emit_consume(pb, pc, pu, pst)

            flat = [(b, c) for b in border for c in range(cb[b])]
            pend = []
            for b, c in flat:
                u, states = emit_score(b, c)
                if len(pend) == 2:
                    pb, pc, pu, pst = pend.pop(0)
                    flush(pb, pc, pu, pst, terminal=False)
                pend.append((b, c, u, states))
            for i, (pb, pc, pu, pst) in enumerate(pend):
                flush(pb, pc, pu, pst, terminal=i == len(pend) - 1)

    nc.compile()
    return nc, L, cb, offs


def _pack_inputs(query, k_cache, v_cache, block_tables, L, cb, offs):
    """Gather the paged cache and pack per-core fp16 shards in device layouts."""
    C = int(offs[-1])
    k_lin = k_cache[block_tables].reshape(B, KV, KVH, D)
    v_lin = v_cache[block_tables].reshape(B, KV, KVH, D)
    kt_all = np.zeros((KVH, D, C * CHUNK), dtype=np.float32)
    v_all = np.zeros((KVH, CHUNK, C * CHUNK), dtype=np.float32)
    for b in range(B):
        Lb, w = int(L[b]), cb[b] * CHUNK
        kk = np.zeros((w, KVH, D), dtype=np.float32)
        kk[:Lb] = k_lin[b, :Lb]
        kt_all[:, :, offs[b] * CHUNK : offs[b] * CHUNK + w] = kk.transpose(
            1, 2, 0
        )
        vv = np.zeros((w, KVH, D), dtype=np.float32)
        vv[:Lb] = v_lin[b, :Lb]
        v_all[:, :, offs[b] * CHUNK : offs[b] * CHUNK + w] = (
            vv.reshape(cb[b], CHUNK, KVH, D)
            .transpose(2, 1, 0, 3)
            .reshape(KVH, CHUNK, w)
        )
    # query [B,Q,H,D] -> [KVH, D, B, Q, G] (t-major, g inner)
    qt_all = (
        query.transpose(2, 3, 0, 1)
        .reshape(KVH, G, D, B, Q)
        .transpose(0, 2, 3, 4, 1)
        .reshape(KVH, D, B * QCOLS)
    )
    kt_all = kt_all.astype(np.float16)
    v_all = v_all.astype(np.float16)
    qt_all = np.ascontiguousarray(qt_all).astype(np.float16)
    return [
        {
            "kt": np.ascontiguousarray(kt_all[h]),
            "v": np.ascontiguousarray(v_all[h]),
            "qt": qt_all[h],
        }
        for h in range(KVH)
    ]


def _unpack_outputs(results):
    """[B,D,QCOLS] fp16 per core (O^T, q=(t,g) on cols) -> [B*Q, H*D] f32."""
    out = np.empty((B * Q, H * D), dtype=np.float32)
    for h, res in enumerate(results):
        o = res["out"].astype(np.float32).reshape(B, D, Q, G)  # [b, d, t, g]
        o = o.transpose(0, 2, 3, 1).reshape(B * Q, G * D)
        out[:, h * G * D : (h + 1) * G * D] = o
    return out


def kernel(query, k_cache, v_cache, block_tables, seq_lens):
    query = np.asarray(query, dtype=np.float32)
    k_cache = np.asarray(k_cache, dtype=np.float32)
    v_cache = np.asarray(v_cache, dtype=np.float32)
    block_tables = np.asarray(block_tables, dtype=np.int64)
    nc, L, cb, offs = _build(np.asarray(seq_lens))
    in_maps = _pack_inputs(query, k_cache, v_cache, block_tables, L, cb, offs)
    res = run_bass_kernel_spmd(nc, in_maps, core_ids=list(range(N_CORES)))
    return _unpack_outputs(res.results)
